# revision 8
# baseline (speedup 1.0000x reference)
"""GAT + 2x GCN message-passing model on 8 Trainium2 NeuronCores.

Sharding: nodes are partitioned across the 8 cores in aligned 1280-row
ranges (10 blocks of 128 dst nodes per core); every edge is owned by the
core that owns its destination node. Weights are replicated; dense per-node
transforms are computed replicated on every core; the per-edge
gather/aggregate work is sharded by dst. Between layers the per-core node
shards are exchanged with AllGather (transposed fp16 layout).

fp16 everywhere on the hot path: gather tables, matmul operands, DVE ops.
Gathers are prefetched one group ahead so the SWDGE DMA overlaps compute.

Self-contained: hardcodes the problem shapes (N=10000, E=320000, IN=128,
HID=64, HEADS=8, OUT=64, neg_slope=0.2).
"""
import math

import numpy as np

# ---------------------------------------------------------------------------
# problem constants
N = 10000
E = 320000
IN_CH = 128
HID = 64
HEADS = 8
OUT_CH = 64
NEG_SLOPE = 0.2

NCORES = 8
NODES_PER_CORE = 1280          # 10 blocks of 128
NB = NODES_PER_CORE // 128     # dst blocks per core
NCHUNK = 79                    # ceil(10000/128) node chunks (rows 0..10111)
NPAD = NCHUNK * 128            # 10112 padded node-table rows
HC = HEADS * HID               # 512
ROW = 640                      # gat table row fp16: a_src(8)|xl(512)|pad(120)
GROW = 128                     # gcn table row fp16: h@W (64) | pad(64)
GRP = 8                        # edge tiles (of 128) per dma_gather call


# ---------------------------------------------------------------------------
# Workaround for walrus codegen 'Too many sync wait commands' on the Tile
# kernel-tail Drain: spread the collected waits one-per-NoOp before the drain.
def _apply_tile_drain_patch():
    import concourse.mybir as mybir
    import concourse.tile as tile_mod
    from concourse.vector_clock import ScopedClock

    if getattr(tile_mod.TileContext, "_drain_patch_applied", False):
        return

    def _patched(self, tick_clock, wait_clock):
        nc = self.nc
        carrier = nc.sync.nop(nofuse=True)
        wait_clock.add_sem_waits(
            carrier.ins, ScopedClock({None: tick_clock.global_clock})
        )
        si = carrier.ins.sync_info
        if si is not None and si.on_wait and len(si.on_wait) > 1:
            waits = list(si.on_wait)
            si.on_wait = waits[:1]
            carrier.ins.sync_info = si
            for w in waits[1:]:
                extra = nc.sync.nop(nofuse=True)
                extra.ins.sync_info = mybir.SyncInfo(on_wait=[w], on_update=[])
        nc.sync.drain()
        nc.all_engine_barrier()
        assert self.sems is not None
        popped = nc._tile_sem_poison_stack.pop()
        assert popped is self._sem_poison
        nc.clear_and_free_semaphores(list(self.sems.allocated().values()))
        nc.all_engine_barrier()

    tile_mod.TileContext._drain_and_barrier = _patched
    tile_mod.TileContext._drain_patch_applied = True


# ---------------------------------------------------------------------------
def _wrap_idx(idx):
    """[n] -> [128, n//16] int16: idx i at [i % 16, i // 16], replicated x8."""
    w = idx.astype(np.int16).reshape(-1, 16).T
    return np.tile(w, (8, 1))


def _host_prep(edge_index):
    """Bucket edges (incl. self loops) by dst block, sort, pad uniformly.

    Returns (Tmax, NGRP, idx_host, dstrel_host) where
      idx_host:    [NCORES, 128, NB*NGRP*64] int16 gather indices
      dstrel_host: [NCORES, 128, NB*Tmax] fp16 (dst - block_base, pad -1000)
    """
    src = np.concatenate([edge_index[0], np.arange(N, dtype=np.int64)])
    dst = np.concatenate([edge_index[1], np.arange(N, dtype=np.int64)])
    order = np.argsort(dst, kind="stable")
    src = src[order]
    dst = dst[order]

    chunk = dst // 128                     # global dst block id, 0..78
    counts = np.bincount(chunk, minlength=NCORES * NB)
    offsets = np.zeros(NCORES * NB + 1, np.int64)
    offsets[1:] = np.cumsum(counts)
    Tmax = max(1, math.ceil(counts.max() / 128))
    NGRP = math.ceil(Tmax / GRP)
    EPAD = NGRP * GRP * 128

    idx_host = np.zeros((NCORES, 128, NB * NGRP * 64), np.int16)
    dstrel_host = np.full((NCORES, 128, NB * Tmax), -1000.0, np.float16)
    for c in range(NCORES):
        for b in range(NB):
            g = c * NB + b
            lo, hi = offsets[g], offsets[g + 1]
            s = np.zeros(EPAD, np.int64)          # pad src = 0 (harmless row)
            s[: hi - lo] = src[lo:hi]
            r = np.full(Tmax * 128, -1000.0, np.float32)
            r[: hi - lo] = (dst[lo:hi] - g * 128).astype(np.float32)
            for gr in range(NGRP):
                idx_host[c, :, (b * NGRP + gr) * 64:(b * NGRP + gr + 1) * 64] = \
                    _wrap_idx(s[gr * GRP * 128:(gr + 1) * GRP * 128])
            dstrel_host[c, :, b * Tmax:(b + 1) * Tmax] = \
                r.reshape(Tmax, 128).T.astype(np.float16)
    return Tmax, NGRP, idx_host, dstrel_host


# ---------------------------------------------------------------------------
def _build_nc(Tmax, NGRP):
    import concourse.bacc as bacc
    import concourse.bass as bass
    import concourse.mybir as mybir
    import concourse.tile as tile

    _apply_tile_drain_patch()
    AO = mybir.AluOpType
    AF = mybir.ActivationFunctionType
    dt = mybir.dt.float32
    f16 = mybir.dt.float16

    nc = bacc.Bacc("TRN2")

    # ---- inputs ----
    xT = nc.dram_tensor("xT", [128, NPAD], f16, kind="ExternalInput")
    xdstT = nc.dram_tensor("xdstT", [128, NODES_PER_CORE], f16, kind="ExternalInput")
    ident_in = nc.dram_tensor("ident_in", [128, 128], f16, kind="ExternalInput")
    iota_in = nc.dram_tensor("iota_in", [1, GRP * 128], f16, kind="ExternalInput")
    gidx = nc.dram_tensor("gidx", [128, NB * NGRP * 64], mybir.dt.int16,
                          kind="ExternalInput")
    dstrel = nc.dram_tensor("dstrel", [128, NB * Tmax], f16, kind="ExternalInput")
    w_gat = nc.dram_tensor("w_gat", [IN_CH, HC], f16, kind="ExternalInput")
    wad = nc.dram_tensor("wad", [IN_CH, 16], f16, kind="ExternalInput")
    wd8 = nc.dram_tensor("wd8", [IN_CH, HEADS], f16, kind="ExternalInput")
    w_emb = nc.dram_tensor("w_emb", [HC, HID], f16, kind="ExternalInput")
    w_g1 = nc.dram_tensor("w_g1", [HID, HID], f16, kind="ExternalInput")
    w_g2 = nc.dram_tensor("w_g2", [HID, OUT_CH], f16, kind="ExternalInput")
    b_gat_r = nc.dram_tensor("b_gat_r", [1, HC], dt, kind="ExternalInput")
    b_emb_r = nc.dram_tensor("b_emb_r", [1, HID], dt, kind="ExternalInput")
    b_g1_r = nc.dram_tensor("b_g1_r", [1, HID], dt, kind="ExternalInput")
    b_g2_r = nc.dram_tensor("b_g2_r", [1, OUT_CH], dt, kind="ExternalInput")
    out = nc.dram_tensor("out", [NODES_PER_CORE, OUT_CH], dt, kind="ExternalOutput")

    gsem = nc.alloc_semaphore("gsem")
    gcount = [0]  # completed dma_gather calls so far

    RG = [list(range(NCORES))]

    def tiles_of(g):
        return min(GRP, Tmax - g * GRP)

    with tile.TileContext(nc) as tc:
        with (
            tc.tile_pool(name="const", bufs=1) as const,
            tc.tile_pool(name="dram", bufs=1, space="DRAM") as dram,
        ):
            # ---- DRAM scratch ----
            gat_table = dram.tile([NPAD, ROW], f16)
            g1_table = dram.tile([NPAD, GROW], f16)
            g2_table = dram.tile([NPAD, GROW], f16)
            h1T_loc = dram.tile([128, NODES_PER_CORE], f16)
            h1T_full = dram.tile([NCORES * 128, NODES_PER_CORE], f16)
            h2T_loc = dram.tile([128, NODES_PER_CORE], f16)
            h2T_full = dram.tile([NCORES * 128, NODES_PER_CORE], f16)

            # ---- constants in SBUF ----
            ident = const.tile([128, 128], f16)
            nc.sync.dma_start(out=ident[:], in_=ident_in[:])
            wgat_t = const.tile([IN_CH, HC], f16)
            nc.sync.dma_start(out=wgat_t[:], in_=w_gat[:])
            wad_t = const.tile([IN_CH, 16], f16)
            nc.sync.dma_start(out=wad_t[:], in_=wad[:])
            wd8_t = const.tile([IN_CH, HEADS], f16)
            nc.sync.dma_start(out=wd8_t[:], in_=wd8[:])
            wemb_t = const.tile([HC // 4, 4, HID], f16)   # [128, 4, 64] chunks
            nc.sync.dma_start(
                out=wemb_t[:],
                in_=w_emb[:].rearrange("(k p) f -> p k f", p=128),
            )
            wg1_t = const.tile([HID, HID], f16)
            nc.sync.dma_start(out=wg1_t[:], in_=w_g1[:])
            wg2_t = const.tile([HID, OUT_CH], f16)
            nc.sync.dma_start(out=wg2_t[:], in_=w_g2[:])
            # bias rows broadcast to 128 partitions (fp32)
            bgat_b = const.tile([128, HC], dt)
            r0 = const.tile([1, HC], dt, tag="r0")
            nc.sync.dma_start(out=r0[:], in_=b_gat_r[:])
            nc.gpsimd.partition_broadcast(bgat_b[:], r0[:1, :])
            bemb_b = const.tile([128, HID], dt)
            r1 = const.tile([1, HID], dt, tag="r1")
            nc.sync.dma_start(out=r1[:], in_=b_emb_r[:])
            nc.gpsimd.partition_broadcast(bemb_b[:], r1[:1, :])
            bg1_b = const.tile([128, HID], dt)
            r2 = const.tile([1, HID], dt, tag="r2")
            nc.sync.dma_start(out=r2[:], in_=b_g1_r[:])
            nc.gpsimd.partition_broadcast(bg1_b[:], r2[:1, :])
            bg2_b = const.tile([128, OUT_CH], dt)
            r3 = const.tile([1, OUT_CH], dt, tag="r3")
            nc.sync.dma_start(out=r3[:], in_=b_g2_r[:])
            nc.gpsimd.partition_broadcast(bg2_b[:], r3[:1, :])
            # iota row constant fp16: m_rows8[p, j] = j % 128
            m_rows8 = const.tile([128, GRP * 128], f16)
            r4 = const.tile([1, GRP * 128], f16, tag="r4")
            nc.sync.dma_start(out=r4[:], in_=iota_in[:])
            nc.gpsimd.partition_broadcast(m_rows8[:], r4[:1, :])
            # resident per-core metadata
            gidx_sb = const.tile([128, NB * NGRP * 64], mybir.dt.int16, tag="gi")
            nc.sync.dma_start(out=gidx_sb[:], in_=gidx[:])
            drt_all = const.tile([128, NB * Tmax], f16, tag="drt")
            nc.sync.dma_start(out=drt_all[:], in_=dstrel[:])
            xT_sb = const.tile([128, NPAD], f16, tag="xT")
            nc.sync.dma_start(out=xT_sb[:], in_=xT[:])
            xdstT_sb = const.tile([128, NODES_PER_CORE], f16, tag="xdT")
            nc.sync.dma_start(out=xdstT_sb[:], in_=xdstT[:])
            # per-core dinv per block, kept across phases
            dinv_all = const.tile([128, NB], dt, tag="dinv")
            # a_dst for own dst windows [128, NB*8] fp16
            adst_all = const.tile([128, NB * HEADS], f16, tag="adst")
            # shared SBUF copy of the allgathered transposed features
            hT_sb = const.tile([128, NCORES, NODES_PER_CORE], f16, tag="hT")

            def elu_inplace(pool, tile_ap, w, dtype):
                """tile_ap [128, w] <- elu(tile_ap); uses pool scratch."""
                xm = pool.tile([128, w], dtype, tag=f"elu{w}")
                nc.vector.tensor_scalar(out=xm[:], in0=tile_ap, scalar1=0.0,
                                        scalar2=None, op0=AO.min)
                nc.scalar.activation(out=xm[:], in_=xm[:], func=AF.Exp)
                nc.vector.tensor_scalar(out=tile_ap, in0=tile_ap, scalar1=0.0,
                                        scalar2=None, op0=AO.max)
                nc.vector.scalar_tensor_tensor(
                    out=tile_ap, in0=tile_ap, scalar=-1.0, in1=xm[:],
                    op0=AO.add, op1=AO.add)

            # =============================================================
            # Phase 0: build gat_table rows [a_src | xl | pad], a_dst windows
            # =============================================================
            with (
                tc.tile_pool(name="p0", bufs=3) as p0,
                tc.tile_pool(name="p0ps", bufs=2, space="PSUM") as p0ps,
            ):
                for k in range(NCHUNK):
                    xl_ps = p0ps.tile([128, HC], dt, tag="xl", space="PSUM")
                    nc.tensor.matmul(out=xl_ps[:],
                                     lhsT=xT_sb[:, 128 * k:128 * (k + 1)],
                                     rhs=wgat_t[:], start=True, stop=True)
                    aw_ps = p0ps.tile([128, 16], dt, tag="aw", space="PSUM")
                    nc.tensor.matmul(out=aw_ps[:],
                                     lhsT=xT_sb[:, 128 * k:128 * (k + 1)],
                                     rhs=wad_t[:], start=True, stop=True)
                    row = p0.tile([128, 8 + HC], f16, tag="row")
                    nc.scalar.copy(out=row[:, 0:8], in_=aw_ps[:, 0:8])
                    nc.vector.tensor_copy(out=row[:, 8:8 + HC], in_=xl_ps[:])
                    nc.sync.dma_start(
                        out=gat_table[128 * k:128 * (k + 1), 0:8 + HC],
                        in_=row[:])
                # a_dst for own windows, from xdstT
                for b in range(NB):
                    ad_ps = p0ps.tile([128, HEADS], dt, tag="aw", space="PSUM")
                    nc.tensor.matmul(out=ad_ps[:],
                                     lhsT=xdstT_sb[:, 128 * b:128 * (b + 1)],
                                     rhs=wd8_t[:], start=True, stop=True)
                    nc.scalar.copy(
                        out=adst_all[:, b * HEADS:(b + 1) * HEADS], in_=ad_ps[:])

            # =============================================================
            # gather prefetch machinery (one shared sem, one gather in
            # flight; the critical section [gather; wait] occupies gpsimd
            # only, so compute on the previous group's tiles overlaps the
            # DMA of the next)
            # =============================================================
            def issue_gather(gpool, table, b, g, row_w, gtag):
                t = tiles_of(g)
                gtile = gpool.tile([128, GRP, row_w], f16, tag=gtag)
                idx0 = (b * NGRP + g) * 64
                with tc.tile_critical(no_gpsimd_drain=True):
                    nc.gpsimd.dma_gather(
                        gtile[:, 0:t, :], table[:],
                        gidx_sb[:, idx0:idx0 + t * 8],
                        t * 128, t * 128, row_w,
                    ).then_inc(gsem, 16)
                    gcount[0] += 1
                    nc.gpsimd.wait_ge(gsem, 16 * gcount[0])
                return gtile

            # =============================================================
            # Phase 1: GAT blocks -> h1T_loc [128, 1280] fp16 (rows 0:64)
            # =============================================================
            with (
                tc.tile_pool(name="pg", bufs=2) as pg,       # gathered rows
                tc.tile_pool(name="pt", bufs=2) as pt,       # per-group scratch
                tc.tile_pool(name="pb", bufs=2) as pb,       # per-block scratch
                tc.tile_pool(name="ps_acc", bufs=2, space="PSUM") as ps_acc,
                tc.tile_pool(name="ps_ed", bufs=2, space="PSUM") as ps_ed,
                tc.tile_pool(name="ps_it", bufs=2, space="PSUM") as ps_it,
            ):
                units = [(b, g) for b in range(NB) for g in range(NGRP)]
                gt_tiles = {}
                gt_tiles[0] = issue_gather(pg, gat_table, 0, 0, ROW, "g")
                blk_state = {}
                for i, (b, g) in enumerate(units):
                    if i + 1 < len(units):
                        nb_, ng_ = units[i + 1]
                        gt_tiles[i + 1] = issue_gather(
                            pg, gat_table, nb_, ng_, ROW, "g")
                    if g == 0:
                        msg_ps = ps_acc.tile([128, HC], dt, tag="msg",
                                             space="PSUM")
                        aux_ps = ps_acc.tile([128, 128], dt, tag="aux",
                                             space="PSUM")
                        blk_state[b] = (msg_ps, aux_ps)
                    msg_ps, aux_ps = blk_state[b]
                    gt = gt_tiles.pop(i)
                    t = tiles_of(g)
                    t0 = g * GRP
                    adw = adst_all[:, b * HEADS:(b + 1) * HEADS]
                    # indicator I for all tiles of the group: [128, t*128] f16
                    I_g = pt.tile([128, GRP * 128], f16, tag="I")
                    nc.vector.tensor_tensor(
                        out=I_g[:, 0:t * 128].rearrange("p (t j) -> p t j", t=t),
                        in0=m_rows8[:, 0:t * 128].rearrange("p (t j) -> p t j", t=t),
                        in1=drt_all[:, b * Tmax + t0:b * Tmax + t0 + t]
                            .to_broadcast([128, t, 128]),
                        op=AO.is_equal)
                    # ed[e, th] = a_dst[dstrel_e, h] via per-tile transpose trick
                    ed_ps = ps_ed.tile([128, GRP * HEADS], dt, tag="ed",
                                       space="PSUM")
                    for tm in range(t):
                        it_ps = ps_it.tile([128, 128], f16, tag="it",
                                           space="PSUM")
                        nc.tensor.transpose(
                            out=it_ps[:],
                            in_=I_g[:, tm * 128:(tm + 1) * 128],
                            identity=ident[:])
                        IT = pt.tile([128, 128], f16, tag="IT")
                        nc.scalar.copy(out=IT[:], in_=it_ps[:])
                        nc.tensor.matmul(
                            out=ed_ps[:, tm * HEADS:(tm + 1) * HEADS],
                            lhsT=IT[:], rhs=adw, start=True, stop=True)
                    # el = leaky(a_src + ed); p = exp(el)
                    el = pt.tile([128, GRP * HEADS], f16, tag="el")
                    nc.vector.tensor_tensor(
                        out=el[:, 0:t * HEADS].rearrange(
                            "p (t h) -> p t h", t=t),
                        in0=gt[:, 0:t, 0:8],
                        in1=ed_ps[:, 0:t * HEADS].rearrange(
                            "p (t h) -> p t h", t=t),
                        op=AO.add)
                    nc.vector.scalar_tensor_tensor(
                        out=el[:, 0:t * HEADS], in0=el[:, 0:t * HEADS],
                        scalar=NEG_SLOPE, in1=el[:, 0:t * HEADS],
                        op0=AO.mult, op1=AO.max)
                    p_blk = pt.tile([128, GRP, 16], f16, tag="p")
                    nc.vector.memset(p_blk[:, :, 8:9], 1.0)
                    nc.scalar.activation(
                        out=p_blk[:, 0:t, 0:8],
                        in_=el[:, 0:t * HEADS].rearrange("p (t h) -> p t h", t=t),
                        func=AF.Exp)
                    # broadcast p over channel dim on ACT, then one big mult
                    p_brd = pt.tile([128, GRP, HEADS, HID], f16, tag="pb")
                    nc.scalar.copy(
                        out=p_brd[:, 0:t],
                        in_=p_blk[:, 0:t, 0:8].to_broadcast([128, t, HEADS, HID]))
                    msg_g = pt.tile([128, GRP, HC], f16, tag="m")
                    nc.vector.tensor_tensor(
                        out=msg_g[:, 0:t].rearrange("p t (h c) -> p t h c", h=HEADS),
                        in0=gt[:, 0:t, 8:8 + HC].rearrange(
                            "p t (h c) -> p t h c", h=HEADS),
                        in1=p_brd[:, 0:t],
                        op=AO.mult)
                    for tm in range(t):
                        tt = t0 + tm
                        nc.tensor.matmul(
                            out=msg_ps[:],
                            lhsT=I_g[:, tm * 128:(tm + 1) * 128],
                            rhs=msg_g[:, tm, :],
                            start=(tt == 0), stop=(tt == Tmax - 1))
                        nc.tensor.matmul(
                            out=aux_ps[:, 0:9],
                            lhsT=I_g[:, tm * 128:(tm + 1) * 128],
                            rhs=p_blk[:, tm, 0:9],
                            start=(tt == 0), stop=(tt == Tmax - 1))
                    if g != NGRP - 1:
                        continue
                    # ---- block epilogue ----
                    del blk_state[b]
                    sinv = pb.tile([128, HEADS], dt, tag="sinv")
                    nc.vector.reciprocal(out=sinv[:], in_=aux_ps[:, 0:8])
                    o = pb.tile([128, HC], f16, tag="o")
                    nc.vector.tensor_tensor(
                        out=o[:].rearrange("p (h c) -> p h c", h=HEADS),
                        in0=msg_ps[:].rearrange("p (h c) -> p h c", h=HEADS),
                        in1=sinv[:].to_broadcast([128, HEADS, HID]),
                        op=AO.mult)
                    nc.vector.tensor_tensor(out=o[:], in0=o[:], in1=bgat_b[:],
                                            op=AO.add)
                    elu_inplace(pb, o[:], HC, f16)
                    # dinv = 1/sqrt(deg)
                    dg = pb.tile([128, 1], dt, tag="dg")
                    nc.scalar.sqrt(out=dg[:], in_=aux_ps[:, 8:9])
                    nc.vector.reciprocal(out=dinv_all[:, b:b + 1], in_=dg[:])
                    # emb: h1 = elu(o @ w_emb + b_emb)
                    emb_ps = aux_ps[:, 64:128]
                    for kk in range(4):
                        ht_ps = ps_it.tile([128, 128], f16, tag="it",
                                           space="PSUM")
                        nc.tensor.transpose(out=ht_ps[:],
                                            in_=o[:, 128 * kk:128 * (kk + 1)],
                                            identity=ident[:])
                        ht = pb.tile([128, 128], f16, tag="ht")
                        nc.scalar.copy(out=ht[:], in_=ht_ps[:])
                        nc.tensor.matmul(out=emb_ps, lhsT=ht[:],
                                         rhs=wemb_t[:, kk, :],
                                         start=(kk == 0), stop=(kk == 3))
                    h1b = pb.tile([128, HID], f16, tag="h1b")
                    nc.vector.tensor_tensor(out=h1b[:], in0=emb_ps,
                                            in1=bemb_b[:], op=AO.add)
                    elu_inplace(pb, h1b[:], HID, f16)
                    # h1d = dinv * h1; transpose; store to h1T_loc block cols
                    nc.vector.tensor_scalar(
                        out=h1b[:], in0=h1b[:], scalar1=dinv_all[:, b:b + 1],
                        scalar2=None, op0=AO.mult)
                    hd_ps = ps_it.tile([128, 128], f16, tag="it",
                                       space="PSUM")
                    nc.tensor.transpose(out=hd_ps[0:HID, :], in_=h1b[:],
                                        identity=ident[:])
                    hdT = pb.tile([HID, 128], f16, tag="hdT")
                    nc.scalar.copy(out=hdT[:], in_=hd_ps[0:HID, :])
                    nc.sync.dma_start(
                        out=h1T_loc[0:HID, 128 * b:128 * (b + 1)], in_=hdT[:])

            # =============================================================
            # GCN layers
            # =============================================================
            nc.gpsimd.collective_compute(
                "AllGather", mybir.AluOpType.bypass, replica_groups=RG,
                ins=[h1T_loc.opt()], outs=[h1T_full.opt()])

            def gcn_layer(hT_full, w_t, bias_b, table, h_out_loc, is_last):
                # table rows = (dinv*h)[n] @ W, from transposed gathered h
                nc.sync.dma_start(
                    out=hT_sb[:],
                    in_=hT_full[:].rearrange("(s p) f -> p s f", p=128))
                with (
                    tc.tile_pool(name="q0", bufs=3) as q0,
                    tc.tile_pool(name="q0ps", bufs=2, space="PSUM") as q0ps,
                ):
                    for k0 in range(0, NCHUNK, 4):
                        kn = min(4, NCHUNK - k0)
                        rows = q0.tile([128, 4, HID], f16, tag="rows")
                        for kk in range(kn):
                            k = k0 + kk
                            c, blk = divmod(k, NB)
                            t_ps = q0ps.tile([128, HID], dt, tag="t",
                                             space="PSUM")
                            nc.tensor.matmul(
                                out=t_ps[:],
                                lhsT=hT_sb[0:HID, c,
                                           128 * blk:128 * (blk + 1)],
                                rhs=w_t[:], start=True, stop=True)
                            nc.scalar.copy(out=rows[:, kk, :], in_=t_ps[:])
                        nc.sync.dma_start(
                            out=table[128 * k0:128 * (k0 + kn), 0:HID]
                                .rearrange("(c p) f -> p c f", p=128),
                            in_=rows[:, 0:kn, :])
                # block loop
                with (
                    tc.tile_pool(name="qg", bufs=2) as qg,
                    tc.tile_pool(name="qt", bufs=2) as qt,
                    tc.tile_pool(name="qb", bufs=2) as qb,
                    tc.tile_pool(name="qps", bufs=2, space="PSUM") as qps,
                ):
                    units = [(b, g) for b in range(NB) for g in range(NGRP)]
                    gt_tiles = {}
                    gt_tiles[0] = issue_gather(qg, table, 0, 0, GROW, "g")
                    blk_ps = {}
                    for i, (b, g) in enumerate(units):
                        if i + 1 < len(units):
                            nb_, ng_ = units[i + 1]
                            gt_tiles[i + 1] = issue_gather(
                                qg, table, nb_, ng_, GROW, "g")
                        if g == 0:
                            blk_ps[b] = qps.tile([128, HID], dt, tag="acc",
                                                 name="accps", space="PSUM")
                        acc_ps = blk_ps[b]
                        gt = gt_tiles.pop(i)
                        t = tiles_of(g)
                        t0 = g * GRP
                        I_g = qt.tile([128, GRP * 128], f16, tag="I")
                        nc.vector.tensor_tensor(
                            out=I_g[:, 0:t * 128].rearrange(
                                "p (t j) -> p t j", t=t),
                            in0=m_rows8[:, 0:t * 128].rearrange(
                                "p (t j) -> p t j", t=t),
                            in1=drt_all[:, b * Tmax + t0:b * Tmax + t0 + t]
                                .to_broadcast([128, t, 128]),
                            op=AO.is_equal)
                        for tm in range(t):
                            tt = t0 + tm
                            nc.tensor.matmul(
                                out=acc_ps[:],
                                lhsT=I_g[:, tm * 128:(tm + 1) * 128],
                                rhs=gt[:, tm, 0:HID],
                                start=(tt == 0), stop=(tt == Tmax - 1))
                        if g != NGRP - 1:
                            continue
                        # epilogue: out = dinv*acc + bias (then elu if mid)
                        del blk_ps[b]
                        if is_last:
                            ob = qb.tile([128, OUT_CH], dt, tag="obf")
                            nc.vector.scalar_tensor_tensor(
                                out=ob[:], in0=acc_ps[:],
                                scalar=dinv_all[:, b:b + 1], in1=bias_b,
                                op0=AO.mult, op1=AO.add)
                            nc.sync.dma_start(
                                out=h_out_loc[128 * b:128 * (b + 1)],
                                in_=ob[:])
                        else:
                            ob = qb.tile([128, HID], f16, tag="ob")
                            nc.vector.scalar_tensor_tensor(
                                out=ob[:], in0=acc_ps[:],
                                scalar=dinv_all[:, b:b + 1], in1=bias_b,
                                op0=AO.mult, op1=AO.add)
                            elu_inplace(qb, ob[:], HID, f16)
                            nc.vector.tensor_scalar(
                                out=ob[:], in0=ob[:],
                                scalar1=dinv_all[:, b:b + 1],
                                scalar2=None, op0=AO.mult)
                            od_ps = qps.tile([HID, 128], f16, tag="od",
                                             space="PSUM")
                            nc.tensor.transpose(out=od_ps[:], in_=ob[:],
                                                identity=ident[:])
                            odT = qb.tile([HID, 128], f16, tag="odT")
                            nc.scalar.copy(out=odT[:], in_=od_ps[:])
                            nc.sync.dma_start(
                                out=h_out_loc[0:HID, 128 * b:128 * (b + 1)],
                                in_=odT[:])

            gcn_layer(h1T_full, wg1_t, bg1_b[:], g1_table, h2T_loc, False)
            nc.gpsimd.collective_compute(
                "AllGather", mybir.AluOpType.bypass, replica_groups=RG,
                ins=[h2T_loc.opt()], outs=[h2T_full.opt()])
            gcn_layer(h2T_full, wg2_t, bg2_b[:], g2_table, out, True)

    nc.finalize()
    return nc


# ---------------------------------------------------------------------------
def _run(inputs, trace=False, **run_kw):
    from concourse import bass_utils

    x = np.asarray(inputs["x"], np.float32)
    edge_index = np.asarray(inputs["edge_index"])
    W_gat = np.asarray(inputs["W_gat"], np.float32)
    att_src = np.asarray(inputs["att_src"], np.float32)
    att_dst = np.asarray(inputs["att_dst"], np.float32)
    b_gat = np.asarray(inputs["b_gat"], np.float32)
    W_emb = np.asarray(inputs["W_emb"], np.float32)
    b_emb = np.asarray(inputs["b_emb"], np.float32)
    W_g1 = np.asarray(inputs["W_g1"], np.float32)
    b_g1 = np.asarray(inputs["b_g1"], np.float32)
    W_g2 = np.asarray(inputs["W_g2"], np.float32)
    b_g2 = np.asarray(inputs["b_g2"], np.float32)

    Tmax, NGRP, idx_host, dstrel_host = _host_prep(edge_index)
    nc = _build_nc(Tmax, NGRP)

    x_big = np.zeros((NCORES * NODES_PER_CORE, IN_CH), np.float32)
    x_big[:N] = x
    xT = x_big[:NPAD].T.astype(np.float16).copy()
    # Wa[k,h] = sum_c W_gat[k, h*HID+c] * att_src[h,c]  (same for Wd/att_dst)
    Wr = W_gat.reshape(IN_CH, HEADS, HID)
    Wa = np.einsum("khc,hc->kh", Wr, att_src).astype(np.float16)
    Wd = np.einsum("khc,hc->kh", Wr, att_dst).astype(np.float16)

    common = {
        "xT": xT,
        "ident_in": np.eye(128, dtype=np.float16),
        "iota_in": (np.arange(GRP * 128) % 128).astype(np.float16)[None, :],
        "w_gat": W_gat.astype(np.float16),
        "wad": np.concatenate([Wa, Wd], axis=1),
        "wd8": Wd,
        "w_emb": W_emb.astype(np.float16),
        "w_g1": W_g1.astype(np.float16),
        "w_g2": W_g2.astype(np.float16),
        "b_gat_r": b_gat[None, :], "b_emb_r": b_emb[None, :],
        "b_g1_r": b_g1[None, :], "b_g2_r": b_g2[None, :],
    }
    in_maps = []
    for c in range(NCORES):
        m = dict(common)
        m["xdstT"] = (
            x_big[c * NODES_PER_CORE:(c + 1) * NODES_PER_CORE].T
            .astype(np.float16).copy())
        m["gidx"] = idx_host[c]
        m["dstrel"] = dstrel_host[c]
        in_maps.append(m)

    res = bass_utils.run_bass_kernel_spmd(
        nc, in_maps, core_ids=list(range(NCORES)), trace=trace, **run_kw)
    full = np.concatenate([res.results[c]["out"] for c in range(NCORES)], axis=0)
    return full[:N], res


def kernel(**inputs) -> np.ndarray:
    out, _ = _run(inputs, trace=False)
    return out


# revision 13
# speedup vs baseline: 1.1440x; 1.1440x over previous
"""GAT + 2x GCN message-passing model on 8 Trainium2 NeuronCores.

Sharding: nodes are partitioned across the 8 cores in aligned 1280-row
ranges (10 blocks of 128 dst nodes per core); every edge is owned by the
core that owns its destination node. Weights are replicated; dense per-node
transforms are computed replicated on every core; the per-edge
gather/aggregate work is sharded by dst. Between layers the per-core node
shards are exchanged with AllGather (transposed fp16 layout).

fp16 everywhere on the hot path: gather tables, matmul operands, DVE ops.
Gathers are prefetched one group ahead so the SWDGE DMA overlaps compute.

Self-contained: hardcodes the problem shapes (N=10000, E=320000, IN=128,
HID=64, HEADS=8, OUT=64, neg_slope=0.2).
"""
import math

import numpy as np

# ---------------------------------------------------------------------------
# problem constants
N = 10000
E = 320000
IN_CH = 128
HID = 64
HEADS = 8
OUT_CH = 64
NEG_SLOPE = 0.2

NCORES = 8
NODES_PER_CORE = 1280          # 10 blocks of 128
NB = NODES_PER_CORE // 128     # dst blocks per core
NCHUNK = 79                    # ceil(10000/128) node chunks (rows 0..10111)
NPAD = NCHUNK * 128            # 10112 padded node-table rows
HC = HEADS * HID               # 512
ROW = 640                      # gat table row fp16: a_src(8)|xl(512)|pad(120)
GROW = 128                     # gcn table row fp16: h@W (64) | pad(64)
GRP = 8                        # edge tiles (of 128) per dma_gather call


# ---------------------------------------------------------------------------
# Workaround for walrus codegen 'Too many sync wait commands' on the Tile
# kernel-tail Drain: spread the collected waits one-per-NoOp before the drain.
def _apply_tile_drain_patch():
    import concourse.mybir as mybir
    import concourse.tile as tile_mod
    from concourse.vector_clock import ScopedClock

    if getattr(tile_mod.TileContext, "_drain_patch_applied", False):
        return

    def _patched(self, tick_clock, wait_clock):
        nc = self.nc
        carrier = nc.sync.nop(nofuse=True)
        wait_clock.add_sem_waits(
            carrier.ins, ScopedClock({None: tick_clock.global_clock})
        )
        si = carrier.ins.sync_info
        if si is not None and si.on_wait and len(si.on_wait) > 1:
            waits = list(si.on_wait)
            si.on_wait = waits[:1]
            carrier.ins.sync_info = si
            for w in waits[1:]:
                extra = nc.sync.nop(nofuse=True)
                extra.ins.sync_info = mybir.SyncInfo(on_wait=[w], on_update=[])
        nc.sync.drain()
        nc.all_engine_barrier()
        assert self.sems is not None
        popped = nc._tile_sem_poison_stack.pop()
        assert popped is self._sem_poison
        nc.clear_and_free_semaphores(list(self.sems.allocated().values()))
        nc.all_engine_barrier()

    tile_mod.TileContext._drain_and_barrier = _patched
    tile_mod.TileContext._drain_patch_applied = True


# ---------------------------------------------------------------------------
def _wrap_idx(idx):
    """[n] -> [128, n//16] int16: idx i at [i % 16, i // 16], replicated x8."""
    w = idx.astype(np.int16).reshape(-1, 16).T
    return np.tile(w, (8, 1))


def _host_prep(edge_index):
    """Bucket edges (incl. self loops) by dst block, sort, pad uniformly.

    Returns (Tmax, NGRP, idx_host, dstrel_host) where
      idx_host:    [NCORES, 128, NB*Tmax*8] int16 gather indices
      dstrel_host: [NCORES, 128, NB*Tmax] fp16 (dst - block_base, pad -1000)
    """
    src = np.concatenate([edge_index[0], np.arange(N, dtype=np.int64)])
    dst = np.concatenate([edge_index[1], np.arange(N, dtype=np.int64)])
    order = np.argsort(dst, kind="stable")
    src = src[order]
    dst = dst[order]

    chunk = dst // 128                     # global dst block id, 0..78
    counts = np.bincount(chunk, minlength=NCORES * NB)
    offsets = np.zeros(NCORES * NB + 1, np.int64)
    offsets[1:] = np.cumsum(counts)
    Tmax = max(1, math.ceil(counts.max() / 128))
    NGRP = math.ceil(Tmax / GRP)
    EPAD = Tmax * 128

    idx_host = np.zeros((NCORES, 128, NB * Tmax * 8), np.int16)
    dstrel_host = np.full((NCORES, 128, NB * Tmax), -1000.0, np.float16)
    for c in range(NCORES):
        for b in range(NB):
            g = c * NB + b
            lo, hi = offsets[g], offsets[g + 1]
            s = np.zeros(EPAD, np.int64)          # pad src = 0 (harmless row)
            s[: hi - lo] = src[lo:hi]
            r = np.full(Tmax * 128, -1000.0, np.float32)
            r[: hi - lo] = (dst[lo:hi] - g * 128).astype(np.float32)
            idx_host[c, :, b * Tmax * 8:(b + 1) * Tmax * 8] = _wrap_idx(s)
            dstrel_host[c, :, b * Tmax:(b + 1) * Tmax] = \
                r.reshape(Tmax, 128).T.astype(np.float16)
    return Tmax, NGRP, idx_host, dstrel_host


# ---------------------------------------------------------------------------
def _build_nc(Tmax, NGRP):
    import concourse.bacc as bacc
    import concourse.bass as bass
    import concourse.mybir as mybir
    import concourse.tile as tile

    _apply_tile_drain_patch()
    AO = mybir.AluOpType
    AF = mybir.ActivationFunctionType
    dt = mybir.dt.float32
    f16 = mybir.dt.float16

    nc = bacc.Bacc("TRN2")

    # ---- inputs ----
    xT = nc.dram_tensor("xT", [128, NPAD], f16, kind="ExternalInput")
    xdstT = nc.dram_tensor("xdstT", [128, NODES_PER_CORE], f16, kind="ExternalInput")
    ident_in = nc.dram_tensor("ident_in", [128, 128], f16, kind="ExternalInput")
    iota_in = nc.dram_tensor("iota_in", [1, GRP * 128], f16, kind="ExternalInput")
    gidx = nc.dram_tensor("gidx", [128, NB * Tmax * 8], mybir.dt.int16,
                          kind="ExternalInput")
    dstrel = nc.dram_tensor("dstrel", [128, NB * Tmax], f16, kind="ExternalInput")
    w_gat = nc.dram_tensor("w_gat", [IN_CH, HC], f16, kind="ExternalInput")
    wad = nc.dram_tensor("wad", [IN_CH, 16], f16, kind="ExternalInput")
    wd8 = nc.dram_tensor("wd8", [IN_CH, HEADS], f16, kind="ExternalInput")
    w_emb = nc.dram_tensor("w_emb", [HC, HID], f16, kind="ExternalInput")
    w_g1 = nc.dram_tensor("w_g1", [HID, HID], f16, kind="ExternalInput")
    w_g2 = nc.dram_tensor("w_g2", [HID, OUT_CH], f16, kind="ExternalInput")
    b_gat_r = nc.dram_tensor("b_gat_r", [1, HC], dt, kind="ExternalInput")
    b_emb_r = nc.dram_tensor("b_emb_r", [1, HID], dt, kind="ExternalInput")
    b_g1_r = nc.dram_tensor("b_g1_r", [1, HID], dt, kind="ExternalInput")
    b_g2_r = nc.dram_tensor("b_g2_r", [1, OUT_CH], dt, kind="ExternalInput")
    out = nc.dram_tensor("out", [NODES_PER_CORE, OUT_CH], dt, kind="ExternalOutput")

    gsem = nc.alloc_semaphore("gsem")
    gcount = [0]  # completed dma_gather calls so far

    RG = [list(range(NCORES))]

    def tiles_of(g):
        return min(GRP, Tmax - g * GRP)

    with tile.TileContext(nc) as tc:
        with (
            tc.tile_pool(name="const", bufs=1) as const,
            tc.tile_pool(name="dram", bufs=1, space="DRAM") as dram,
        ):
            # ---- DRAM scratch ----
            gat_table = dram.tile([NPAD, ROW], f16)
            g1_table = dram.tile([NPAD, GROW], f16)
            g2_table = dram.tile([NPAD, GROW], f16)
            h1T_loc = dram.tile([128, NODES_PER_CORE], f16)
            h1T_full = dram.tile([NCORES * 128, NODES_PER_CORE], f16)
            h2T_loc = dram.tile([128, NODES_PER_CORE], f16)
            h2T_full = dram.tile([NCORES * 128, NODES_PER_CORE], f16)

            # ---- constants in SBUF ----
            ident = const.tile([128, 128], f16)
            nc.sync.dma_start(out=ident[:], in_=ident_in[:])
            wgat_t = const.tile([IN_CH, HC], f16)
            nc.sync.dma_start(out=wgat_t[:], in_=w_gat[:])
            wad_t = const.tile([IN_CH, 16], f16)
            nc.sync.dma_start(out=wad_t[:], in_=wad[:])
            wd8_t = const.tile([IN_CH, HEADS], f16)
            nc.sync.dma_start(out=wd8_t[:], in_=wd8[:])
            wemb_t = const.tile([HC // 4, 4, HID], f16)   # [128, 4, 64] chunks
            nc.sync.dma_start(
                out=wemb_t[:],
                in_=w_emb[:].rearrange("(k p) f -> p k f", p=128),
            )
            wg1_t = const.tile([HID, HID], f16)
            nc.sync.dma_start(out=wg1_t[:], in_=w_g1[:])
            wg2_t = const.tile([HID, OUT_CH], f16)
            nc.sync.dma_start(out=wg2_t[:], in_=w_g2[:])
            # bias rows broadcast to 128 partitions (fp32)
            bgat_b = const.tile([128, HC], dt)
            r0 = const.tile([1, HC], dt, tag="r0")
            nc.sync.dma_start(out=r0[:], in_=b_gat_r[:])
            nc.gpsimd.partition_broadcast(bgat_b[:], r0[:1, :])
            bemb_b = const.tile([128, HID], dt)
            r1 = const.tile([1, HID], dt, tag="r1")
            nc.sync.dma_start(out=r1[:], in_=b_emb_r[:])
            nc.gpsimd.partition_broadcast(bemb_b[:], r1[:1, :])
            bg1_b = const.tile([128, HID], dt)
            r2 = const.tile([1, HID], dt, tag="r2")
            nc.sync.dma_start(out=r2[:], in_=b_g1_r[:])
            nc.gpsimd.partition_broadcast(bg1_b[:], r2[:1, :])
            bg2_b = const.tile([128, OUT_CH], dt)
            r3 = const.tile([1, OUT_CH], dt, tag="r3")
            nc.sync.dma_start(out=r3[:], in_=b_g2_r[:])
            nc.gpsimd.partition_broadcast(bg2_b[:], r3[:1, :])
            # iota row constant fp16: m_rows8[p, j] = j % 128
            m_rows8 = const.tile([128, GRP * 128], f16)
            r4 = const.tile([1, GRP * 128], f16, tag="r4")
            nc.sync.dma_start(out=r4[:], in_=iota_in[:])
            nc.gpsimd.partition_broadcast(m_rows8[:], r4[:1, :])
            # resident per-core metadata
            gidx_sb = const.tile([128, NB * Tmax * 8], mybir.dt.int16, tag="gi")
            nc.sync.dma_start(out=gidx_sb[:], in_=gidx[:])
            drt_all = const.tile([128, NB * Tmax], f16, tag="drt")
            nc.sync.dma_start(out=drt_all[:], in_=dstrel[:])
            # per-core dinv per block, kept across phases
            dinv_all = const.tile([128, NB], dt, tag="dinv")
            # a_dst for own dst windows [128, NB*8] fp16
            adst_all = const.tile([128, NB * HEADS], f16, tag="adst")
            # shared SBUF copy of the allgathered transposed features
            hT_sb = const.tile([128, NCORES, NODES_PER_CORE], f16, tag="hT")

            def elu_inplace(pool, tile_ap, w, dtype):
                """tile_ap [128, w] <- elu(tile_ap); uses pool scratch."""
                xm = pool.tile([128, w], dtype, tag=f"elu{w}")
                nc.vector.tensor_scalar(out=xm[:], in0=tile_ap, scalar1=0.0,
                                        scalar2=None, op0=AO.min)
                nc.scalar.activation(out=xm[:], in_=xm[:], func=AF.Exp)
                nc.vector.tensor_scalar(out=tile_ap, in0=tile_ap, scalar1=0.0,
                                        scalar2=None, op0=AO.max)
                nc.vector.scalar_tensor_tensor(
                    out=tile_ap, in0=tile_ap, scalar=-1.0, in1=xm[:],
                    op0=AO.add, op1=AO.add)

            # =============================================================
            # Phase 0: build gat_table rows [a_src | xl | pad], a_dst windows
            # =============================================================
            with (
                tc.tile_pool(name="p0c", bufs=1) as p0c,
                tc.tile_pool(name="p0", bufs=3) as p0,
                tc.tile_pool(name="p0ps", bufs=2, space="PSUM") as p0ps,
            ):
                xT_sb = p0c.tile([128, NPAD], f16, tag="xT")
                nc.sync.dma_start(out=xT_sb[:], in_=xT[:])
                xdstT_sb = p0c.tile([128, NODES_PER_CORE], f16, tag="xdT")
                nc.sync.dma_start(out=xdstT_sb[:], in_=xdstT[:])
                for k in range(NCHUNK):
                    xl_ps = p0ps.tile([128, HC], dt, tag="xl", space="PSUM")
                    nc.tensor.matmul(out=xl_ps[:],
                                     lhsT=xT_sb[:, 128 * k:128 * (k + 1)],
                                     rhs=wgat_t[:], start=True, stop=True)
                    aw_ps = p0ps.tile([128, 16], dt, tag="aw", space="PSUM")
                    nc.tensor.matmul(out=aw_ps[:],
                                     lhsT=xT_sb[:, 128 * k:128 * (k + 1)],
                                     rhs=wad_t[:], start=True, stop=True)
                    row = p0.tile([128, 8 + HC], f16, tag="row")
                    nc.scalar.copy(out=row[:, 0:8], in_=aw_ps[:, 0:8])
                    nc.vector.tensor_copy(out=row[:, 8:8 + HC], in_=xl_ps[:])
                    nc.sync.dma_start(
                        out=gat_table[128 * k:128 * (k + 1), 0:8 + HC],
                        in_=row[:])
                # a_dst for own windows, from xdstT
                for b in range(NB):
                    ad_ps = p0ps.tile([128, HEADS], dt, tag="aw", space="PSUM")
                    nc.tensor.matmul(out=ad_ps[:],
                                     lhsT=xdstT_sb[:, 128 * b:128 * (b + 1)],
                                     rhs=wd8_t[:], start=True, stop=True)
                    nc.scalar.copy(
                        out=adst_all[:, b * HEADS:(b + 1) * HEADS], in_=ad_ps[:])

            # =============================================================
            # gather prefetch machinery (one shared sem, one gather in
            # flight; the critical section [gather; wait] occupies gpsimd
            # only, so compute on the previous group's tiles overlaps the
            # DMA of the next)
            # =============================================================
            def issue_gather(gpool, table, b, row_w, gtag):
                """One gather per dst block: all Tmax*128 edge rows."""
                gtile = gpool.tile([128, Tmax, row_w], f16, tag=gtag)
                idx0 = b * Tmax * 8
                with tc.tile_critical(no_gpsimd_drain=True):
                    nc.gpsimd.dma_gather(
                        gtile[:], table[:],
                        gidx_sb[:, idx0:idx0 + Tmax * 8],
                        Tmax * 128, Tmax * 128, row_w,
                        single_packet=False,
                    ).then_inc(gsem, 16)
                    gcount[0] += 1
                    nc.gpsimd.wait_ge(gsem, 16 * gcount[0])
                return gtile

            # =============================================================
            # Phase 1: GAT blocks -> h1T_loc [128, 1280] fp16 (rows 0:64)
            # =============================================================
            with (
                tc.tile_pool(name="pg", bufs=2) as pg,       # gathered rows
                tc.tile_pool(name="pt", bufs=2) as pt,       # per-group scratch
                tc.tile_pool(name="pb", bufs=2) as pb,       # per-block scratch
                tc.tile_pool(name="ps_acc", bufs=2, space="PSUM") as ps_acc,
                tc.tile_pool(name="ps_ed", bufs=2, space="PSUM") as ps_ed,
                tc.tile_pool(name="ps_it", bufs=2, space="PSUM") as ps_it,
            ):
                gt_next = issue_gather(pg, gat_table, 0, ROW, "g")
                for b in range(NB):
                    gt = gt_next
                    if b + 1 < NB:
                        gt_next = issue_gather(pg, gat_table, b + 1, ROW, "g")
                    msg_ps = ps_acc.tile([128, HC], dt, tag="msg",
                                         space="PSUM")
                    aux_ps = ps_acc.tile([128, 128], dt, tag="aux",
                                         space="PSUM")
                    adw = adst_all[:, b * HEADS:(b + 1) * HEADS]
                    for g in range(NGRP):
                        t = tiles_of(g)
                        t0 = g * GRP
                        gts = gt[:, t0:t0 + t, :]
                        # indicator I for the sub-group: [128, t*128] f16
                        I_g = pt.tile([128, GRP * 128], f16, tag="I")
                        nc.vector.tensor_tensor(
                            out=I_g[:, 0:t * 128].rearrange(
                                "p (t j) -> p t j", t=t),
                            in0=m_rows8[:, 0:t * 128].rearrange(
                                "p (t j) -> p t j", t=t),
                            in1=drt_all[:, b * Tmax + t0:b * Tmax + t0 + t]
                                .to_broadcast([128, t, 128]),
                            op=AO.is_equal)
                        # ed[e, th] = a_dst[dstrel_e, h] via transpose trick
                        ed_ps = ps_ed.tile([128, GRP * HEADS], dt, tag="ed",
                                           space="PSUM")
                        for tm in range(t):
                            it_ps = ps_it.tile([128, 128], f16, tag="it",
                                               space="PSUM")
                            nc.tensor.transpose(
                                out=it_ps[:],
                                in_=I_g[:, tm * 128:(tm + 1) * 128],
                                identity=ident[:])
                            IT = pt.tile([128, 128], f16, tag="IT")
                            nc.scalar.copy(out=IT[:], in_=it_ps[:])
                            nc.tensor.matmul(
                                out=ed_ps[:, tm * HEADS:(tm + 1) * HEADS],
                                lhsT=IT[:], rhs=adw, start=True, stop=True)
                        # el = leaky(a_src + ed); p = exp(el)
                        el = pt.tile([128, GRP * HEADS], f16, tag="el")
                        nc.vector.tensor_tensor(
                            out=el[:, 0:t * HEADS].rearrange(
                                "p (t h) -> p t h", t=t),
                            in0=gts[:, :, 0:8],
                            in1=ed_ps[:, 0:t * HEADS].rearrange(
                                "p (t h) -> p t h", t=t),
                            op=AO.add)
                        nc.vector.scalar_tensor_tensor(
                            out=el[:, 0:t * HEADS], in0=el[:, 0:t * HEADS],
                            scalar=NEG_SLOPE, in1=el[:, 0:t * HEADS],
                            op0=AO.mult, op1=AO.max)
                        p_blk = pt.tile([128, GRP, 16], f16, tag="p")
                        nc.vector.memset(p_blk[:, :, 8:9], 1.0)
                        nc.scalar.activation(
                            out=p_blk[:, 0:t, 0:8],
                            in_=el[:, 0:t * HEADS].rearrange(
                                "p (t h) -> p t h", t=t),
                            func=AF.Exp)
                        # broadcast p over channel dim on ACT, one big mult
                        p_brd = pt.tile([128, GRP, HEADS, HID], f16, tag="pb")
                        nc.scalar.copy(
                            out=p_brd[:, 0:t],
                            in_=p_blk[:, 0:t, 0:8].to_broadcast(
                                [128, t, HEADS, HID]))
                        msg_g = pt.tile([128, GRP, HC], f16, tag="m")
                        nc.vector.tensor_tensor(
                            out=msg_g[:, 0:t].rearrange(
                                "p t (h c) -> p t h c", h=HEADS),
                            in0=gts[:, :, 8:8 + HC].rearrange(
                                "p t (h c) -> p t h c", h=HEADS),
                            in1=p_brd[:, 0:t],
                            op=AO.mult)
                        for tm in range(t):
                            tt = t0 + tm
                            nc.tensor.matmul(
                                out=msg_ps[:],
                                lhsT=I_g[:, tm * 128:(tm + 1) * 128],
                                rhs=msg_g[:, tm, :],
                                start=(tt == 0), stop=(tt == Tmax - 1))
                            nc.tensor.matmul(
                                out=aux_ps[:, 0:9],
                                lhsT=I_g[:, tm * 128:(tm + 1) * 128],
                                rhs=p_blk[:, tm, 0:9],
                                start=(tt == 0), stop=(tt == Tmax - 1))
                    # ---- block epilogue ----
                    sinv = pb.tile([128, HEADS], dt, tag="sinv")
                    nc.vector.reciprocal(out=sinv[:], in_=aux_ps[:, 0:8])
                    o = pb.tile([128, HC], f16, tag="o")
                    nc.vector.tensor_tensor(
                        out=o[:].rearrange("p (h c) -> p h c", h=HEADS),
                        in0=msg_ps[:].rearrange("p (h c) -> p h c", h=HEADS),
                        in1=sinv[:].to_broadcast([128, HEADS, HID]),
                        op=AO.mult)
                    nc.vector.tensor_tensor(out=o[:], in0=o[:], in1=bgat_b[:],
                                            op=AO.add)
                    elu_inplace(pb, o[:], HC, f16)
                    # dinv = 1/sqrt(deg)
                    dg = pb.tile([128, 1], dt, tag="dg")
                    nc.scalar.sqrt(out=dg[:], in_=aux_ps[:, 8:9])
                    nc.vector.reciprocal(out=dinv_all[:, b:b + 1], in_=dg[:])
                    # emb: h1 = elu(o @ w_emb + b_emb)
                    emb_ps = aux_ps[:, 64:128]
                    for kk in range(4):
                        ht_ps = ps_it.tile([128, 128], f16, tag="it",
                                           space="PSUM")
                        nc.tensor.transpose(out=ht_ps[:],
                                            in_=o[:, 128 * kk:128 * (kk + 1)],
                                            identity=ident[:])
                        ht = pb.tile([128, 128], f16, tag="ht")
                        nc.scalar.copy(out=ht[:], in_=ht_ps[:])
                        nc.tensor.matmul(out=emb_ps, lhsT=ht[:],
                                         rhs=wemb_t[:, kk, :],
                                         start=(kk == 0), stop=(kk == 3))
                    h1b = pb.tile([128, HID], f16, tag="h1b")
                    nc.vector.tensor_tensor(out=h1b[:], in0=emb_ps,
                                            in1=bemb_b[:], op=AO.add)
                    elu_inplace(pb, h1b[:], HID, f16)
                    # h1d = dinv * h1; transpose; store to h1T_loc block cols
                    nc.vector.tensor_scalar(
                        out=h1b[:], in0=h1b[:], scalar1=dinv_all[:, b:b + 1],
                        scalar2=None, op0=AO.mult)
                    hd_ps = ps_it.tile([128, 128], f16, tag="it",
                                       space="PSUM")
                    nc.tensor.transpose(out=hd_ps[0:HID, :], in_=h1b[:],
                                        identity=ident[:])
                    hdT = pb.tile([HID, 128], f16, tag="hdT")
                    nc.scalar.copy(out=hdT[:], in_=hd_ps[0:HID, :])
                    nc.sync.dma_start(
                        out=h1T_loc[0:HID, 128 * b:128 * (b + 1)], in_=hdT[:])

            # =============================================================
            # GCN layers
            # =============================================================
            nc.gpsimd.collective_compute(
                "AllGather", mybir.AluOpType.bypass, replica_groups=RG,
                ins=[h1T_loc.opt()], outs=[h1T_full.opt()])

            def gcn_layer(hT_full, w_t, bias_b, table, h_out_loc, is_last):
                # table rows = (dinv*h)[n] @ W, from transposed gathered h
                nc.sync.dma_start(
                    out=hT_sb[:],
                    in_=hT_full[:].rearrange("(s p) f -> p s f", p=128))
                with (
                    tc.tile_pool(name="q0", bufs=3) as q0,
                    tc.tile_pool(name="q0ps", bufs=2, space="PSUM") as q0ps,
                ):
                    for k0 in range(0, NCHUNK, 4):
                        kn = min(4, NCHUNK - k0)
                        rows = q0.tile([128, 4, HID], f16, tag="rows")
                        for kk in range(kn):
                            k = k0 + kk
                            c, blk = divmod(k, NB)
                            t_ps = q0ps.tile([128, HID], dt, tag="t",
                                             space="PSUM")
                            nc.tensor.matmul(
                                out=t_ps[:],
                                lhsT=hT_sb[0:HID, c,
                                           128 * blk:128 * (blk + 1)],
                                rhs=w_t[:], start=True, stop=True)
                            nc.scalar.copy(out=rows[:, kk, :], in_=t_ps[:])
                        nc.sync.dma_start(
                            out=table[128 * k0:128 * (k0 + kn), 0:HID]
                                .rearrange("(c p) f -> p c f", p=128),
                            in_=rows[:, 0:kn, :])
                # block loop
                with (
                    tc.tile_pool(name="qg", bufs=2) as qg,
                    tc.tile_pool(name="qt", bufs=2) as qt,
                    tc.tile_pool(name="qb", bufs=2) as qb,
                    tc.tile_pool(name="qps", bufs=2, space="PSUM") as qps,
                ):
                    gt_next = issue_gather(qg, table, 0, GROW, "g")
                    for b in range(NB):
                        gt = gt_next
                        if b + 1 < NB:
                            gt_next = issue_gather(qg, table, b + 1, GROW, "g")
                        acc_ps = qps.tile([128, HID], dt, tag="acc",
                                          name="accps", space="PSUM")
                        for g in range(NGRP):
                            t = tiles_of(g)
                            t0 = g * GRP
                            I_g = qt.tile([128, GRP * 128], f16, tag="I")
                            nc.vector.tensor_tensor(
                                out=I_g[:, 0:t * 128].rearrange(
                                    "p (t j) -> p t j", t=t),
                                in0=m_rows8[:, 0:t * 128].rearrange(
                                    "p (t j) -> p t j", t=t),
                                in1=drt_all[:, b * Tmax + t0:b * Tmax + t0 + t]
                                    .to_broadcast([128, t, 128]),
                                op=AO.is_equal)
                            for tm in range(t):
                                tt = t0 + tm
                                nc.tensor.matmul(
                                    out=acc_ps[:],
                                    lhsT=I_g[:, tm * 128:(tm + 1) * 128],
                                    rhs=gt[:, t0 + tm, 0:HID],
                                    start=(tt == 0), stop=(tt == Tmax - 1))
                        # epilogue: out = dinv*acc + bias (then elu if mid)
                        if is_last:
                            ob = qb.tile([128, OUT_CH], dt, tag="obf")
                            nc.vector.scalar_tensor_tensor(
                                out=ob[:], in0=acc_ps[:],
                                scalar=dinv_all[:, b:b + 1], in1=bias_b,
                                op0=AO.mult, op1=AO.add)
                            nc.sync.dma_start(
                                out=h_out_loc[128 * b:128 * (b + 1)],
                                in_=ob[:])
                        else:
                            ob = qb.tile([128, HID], f16, tag="ob")
                            nc.vector.scalar_tensor_tensor(
                                out=ob[:], in0=acc_ps[:],
                                scalar=dinv_all[:, b:b + 1], in1=bias_b,
                                op0=AO.mult, op1=AO.add)
                            elu_inplace(qb, ob[:], HID, f16)
                            nc.vector.tensor_scalar(
                                out=ob[:], in0=ob[:],
                                scalar1=dinv_all[:, b:b + 1],
                                scalar2=None, op0=AO.mult)
                            od_ps = qps.tile([HID, 128], f16, tag="od",
                                             space="PSUM")
                            nc.tensor.transpose(out=od_ps[:], in_=ob[:],
                                                identity=ident[:])
                            odT = qb.tile([HID, 128], f16, tag="odT")
                            nc.scalar.copy(out=odT[:], in_=od_ps[:])
                            nc.sync.dma_start(
                                out=h_out_loc[0:HID, 128 * b:128 * (b + 1)],
                                in_=odT[:])

            gcn_layer(h1T_full, wg1_t, bg1_b[:], g1_table, h2T_loc, False)
            nc.gpsimd.collective_compute(
                "AllGather", mybir.AluOpType.bypass, replica_groups=RG,
                ins=[h2T_loc.opt()], outs=[h2T_full.opt()])
            gcn_layer(h2T_full, wg2_t, bg2_b[:], g2_table, out, True)

    nc.finalize()
    return nc


# ---------------------------------------------------------------------------
def _run(inputs, trace=False, **run_kw):
    from concourse import bass_utils

    x = np.asarray(inputs["x"], np.float32)
    edge_index = np.asarray(inputs["edge_index"])
    W_gat = np.asarray(inputs["W_gat"], np.float32)
    att_src = np.asarray(inputs["att_src"], np.float32)
    att_dst = np.asarray(inputs["att_dst"], np.float32)
    b_gat = np.asarray(inputs["b_gat"], np.float32)
    W_emb = np.asarray(inputs["W_emb"], np.float32)
    b_emb = np.asarray(inputs["b_emb"], np.float32)
    W_g1 = np.asarray(inputs["W_g1"], np.float32)
    b_g1 = np.asarray(inputs["b_g1"], np.float32)
    W_g2 = np.asarray(inputs["W_g2"], np.float32)
    b_g2 = np.asarray(inputs["b_g2"], np.float32)

    Tmax, NGRP, idx_host, dstrel_host = _host_prep(edge_index)
    nc = _build_nc(Tmax, NGRP)

    x_big = np.zeros((NCORES * NODES_PER_CORE, IN_CH), np.float32)
    x_big[:N] = x
    xT = x_big[:NPAD].T.astype(np.float16).copy()
    # Wa[k,h] = sum_c W_gat[k, h*HID+c] * att_src[h,c]  (same for Wd/att_dst)
    Wr = W_gat.reshape(IN_CH, HEADS, HID)
    Wa = np.einsum("khc,hc->kh", Wr, att_src).astype(np.float16)
    Wd = np.einsum("khc,hc->kh", Wr, att_dst).astype(np.float16)

    common = {
        "xT": xT,
        "ident_in": np.eye(128, dtype=np.float16),
        "iota_in": (np.arange(GRP * 128) % 128).astype(np.float16)[None, :],
        "w_gat": W_gat.astype(np.float16),
        "wad": np.concatenate([Wa, Wd], axis=1),
        "wd8": Wd,
        "w_emb": W_emb.astype(np.float16),
        "w_g1": W_g1.astype(np.float16),
        "w_g2": W_g2.astype(np.float16),
        "b_gat_r": b_gat[None, :], "b_emb_r": b_emb[None, :],
        "b_g1_r": b_g1[None, :], "b_g2_r": b_g2[None, :],
    }
    in_maps = []
    for c in range(NCORES):
        m = dict(common)
        m["xdstT"] = (
            x_big[c * NODES_PER_CORE:(c + 1) * NODES_PER_CORE].T
            .astype(np.float16).copy())
        m["gidx"] = idx_host[c]
        m["dstrel"] = dstrel_host[c]
        in_maps.append(m)

    res = bass_utils.run_bass_kernel_spmd(
        nc, in_maps, core_ids=list(range(NCORES)), trace=trace, **run_kw)
    full = np.concatenate([res.results[c]["out"] for c in range(NCORES)], axis=0)
    return full[:N], res


def kernel(**inputs) -> np.ndarray:
    out, _ = _run(inputs, trace=False)
    return out


# revision 14
# speedup vs baseline: 2.0432x; 1.7860x over previous
"""GAT + 2x GCN message-passing model on 8 Trainium2 NeuronCores.

Sharding: nodes are partitioned across the 8 cores in aligned 1280-row
ranges (10 blocks of 128 dst nodes per core); every edge is owned by the
core that owns its destination node. Weights are replicated.

GAT layer: per-edge dma_gather of fp16 [a_src | xl] rows from a
replicated node table (one gather per dst block, prefetched one block
ahead so the SWDGE descriptor generation overlaps compute), attention
via batched fp16 DVE ops + indicator-matmul scatter.

GCN layers: fully dense — the per-edge weight dinv[src]*dinv[dst] is
host-known, so the host ships a blocked A^T matrix (fp16) and each layer
is a streamed dense matmul A_b @ (h @ W); no gathers at all. The h@W
table lives in SBUF. Between layers the per-core node shards are
exchanged with AllGather in a transposed fp16 layout.

Self-contained: hardcodes the problem shapes (N=10000, E=320000, IN=128,
HID=64, HEADS=8, OUT=64, neg_slope=0.2).
"""
import math

import numpy as np

# ---------------------------------------------------------------------------
# problem constants
N = 10000
E = 320000
IN_CH = 128
HID = 64
HEADS = 8
OUT_CH = 64
NEG_SLOPE = 0.2

NCORES = 8
NODES_PER_CORE = 1280          # 10 blocks of 128
NB = NODES_PER_CORE // 128     # dst blocks per core
NCHUNK = 79                    # ceil(10000/128) node chunks (rows 0..10111)
NPAD = NCHUNK * 128            # 10112 padded node-table rows
HC = HEADS * HID               # 512
ROW = 640                      # gat table row fp16: a_src(8)|xl(512)|pad(120)
GRP = 8                        # edge tiles (of 128) per compute sub-group


# ---------------------------------------------------------------------------
# Workaround for walrus codegen 'Too many sync wait commands' on the Tile
# kernel-tail Drain: spread the collected waits one-per-NoOp before the drain.
def _apply_tile_drain_patch():
    import concourse.mybir as mybir
    import concourse.tile as tile_mod
    from concourse.vector_clock import ScopedClock

    if getattr(tile_mod.TileContext, "_drain_patch_applied", False):
        return

    def _patched(self, tick_clock, wait_clock):
        nc = self.nc
        carrier = nc.sync.nop(nofuse=True)
        wait_clock.add_sem_waits(
            carrier.ins, ScopedClock({None: tick_clock.global_clock})
        )
        si = carrier.ins.sync_info
        if si is not None and si.on_wait and len(si.on_wait) > 1:
            waits = list(si.on_wait)
            si.on_wait = waits[:1]
            carrier.ins.sync_info = si
            for w in waits[1:]:
                extra = nc.sync.nop(nofuse=True)
                extra.ins.sync_info = mybir.SyncInfo(on_wait=[w], on_update=[])
        nc.sync.drain()
        nc.all_engine_barrier()
        assert self.sems is not None
        popped = nc._tile_sem_poison_stack.pop()
        assert popped is self._sem_poison
        nc.clear_and_free_semaphores(list(self.sems.allocated().values()))
        nc.all_engine_barrier()

    tile_mod.TileContext._drain_and_barrier = _patched
    tile_mod.TileContext._drain_patch_applied = True


# ---------------------------------------------------------------------------
def _wrap_idx(idx):
    """[n] -> [128, n//16] int16: idx i at [i % 16, i // 16], replicated x8."""
    w = idx.astype(np.int16).reshape(-1, 16).T
    return np.tile(w, (8, 1))


def _host_prep(edge_index):
    """Edge prep: GAT gather indices + dstrel, dense GCN A^T blocks.

    Returns (Tmax, NGRP, idx_host, dstrel_host, a_host) where
      idx_host:    [NCORES, 128, NB*Tmax*8] int16 gather indices
      dstrel_host: [NCORES, 128, NB*Tmax] fp16 (dst - block_base, pad -1000)
      a_host:      [NCORES, NB, 128, NCHUNK*128] fp16
                   a_host[c, b, j, k*128+d... layout [j, (k d)]] = A^T with
                   A[dst, src] = mult * dinv[src] * dinv[dst] (incl. self loop)
    """
    src = np.concatenate([edge_index[0], np.arange(N, dtype=np.int64)])
    dst = np.concatenate([edge_index[1], np.arange(N, dtype=np.int64)])

    # degrees incl. self loop, from dst counts
    deg = np.bincount(dst, minlength=N).astype(np.float32)
    dinv = 1.0 / np.sqrt(deg)

    # dense A^T blocks: flat[dst_block, src_chunk, src_rel, dst_rel]
    a_flat = np.zeros(NCORES * NB * NCHUNK * 128 * 128, np.float32)
    db = dst // 128
    sc = src // 128
    sr = src % 128
    dr = dst % 128
    lin = ((db * NCHUNK + sc) * 128 + sr) * 128 + dr
    np.add.at(a_flat, lin, dinv[src] * dinv[dst])
    a_all = a_flat.reshape(NCORES, NB, NCHUNK, 128, 128).astype(np.float16)
    # rearrange to [c, b, j(src_rel), chunk, dst_rel] for direct lhsT slices
    a_host = np.ascontiguousarray(a_all.transpose(0, 1, 3, 2, 4)).reshape(
        NCORES, NB, 128, NCHUNK * 128)

    order = np.argsort(dst, kind="stable")
    src_s = src[order]
    dst_s = dst[order]
    chunk = dst_s // 128
    counts = np.bincount(chunk, minlength=NCORES * NB)
    offsets = np.zeros(NCORES * NB + 1, np.int64)
    offsets[1:] = np.cumsum(counts)
    Tmax = max(1, math.ceil(counts.max() / 128))
    NGRP = math.ceil(Tmax / GRP)
    EPAD = Tmax * 128

    idx_host = np.zeros((NCORES, 128, NB * Tmax * 8), np.int16)
    dstrel_host = np.full((NCORES, 128, NB * Tmax), -1000.0, np.float16)
    for c in range(NCORES):
        for b in range(NB):
            g = c * NB + b
            lo, hi = offsets[g], offsets[g + 1]
            s = np.zeros(EPAD, np.int64)          # pad src = 0 (harmless row)
            s[: hi - lo] = src_s[lo:hi]
            r = np.full(Tmax * 128, -1000.0, np.float32)
            r[: hi - lo] = (dst_s[lo:hi] - g * 128).astype(np.float32)
            idx_host[c, :, b * Tmax * 8:(b + 1) * Tmax * 8] = _wrap_idx(s)
            dstrel_host[c, :, b * Tmax:(b + 1) * Tmax] = \
                r.reshape(Tmax, 128).T.astype(np.float16)
    return Tmax, NGRP, idx_host, dstrel_host, a_host


# ---------------------------------------------------------------------------
def _build_nc(Tmax, NGRP):
    import concourse.bacc as bacc
    import concourse.mybir as mybir
    import concourse.tile as tile

    _apply_tile_drain_patch()
    AO = mybir.AluOpType
    AF = mybir.ActivationFunctionType
    dt = mybir.dt.float32
    f16 = mybir.dt.float16

    nc = bacc.Bacc("TRN2")

    # ---- inputs ----
    xT = nc.dram_tensor("xT", [128, NPAD], f16, kind="ExternalInput")
    xdstT = nc.dram_tensor("xdstT", [128, NODES_PER_CORE], f16,
                           kind="ExternalInput")
    ident_in = nc.dram_tensor("ident_in", [128, 128], f16, kind="ExternalInput")
    iota_in = nc.dram_tensor("iota_in", [1, GRP * 128], f16,
                             kind="ExternalInput")
    gidx = nc.dram_tensor("gidx", [128, NB * Tmax * 8], mybir.dt.int16,
                          kind="ExternalInput")
    dstrel = nc.dram_tensor("dstrel", [128, NB * Tmax], f16,
                            kind="ExternalInput")
    a_gcn = nc.dram_tensor("a_gcn", [NB, 128, NCHUNK * 128], f16,
                           kind="ExternalInput")
    w_gat = nc.dram_tensor("w_gat", [IN_CH, HC], f16, kind="ExternalInput")
    wad = nc.dram_tensor("wad", [IN_CH, 16], f16, kind="ExternalInput")
    wd8 = nc.dram_tensor("wd8", [IN_CH, HEADS], f16, kind="ExternalInput")
    w_emb = nc.dram_tensor("w_emb", [HC, HID], f16, kind="ExternalInput")
    w_g1 = nc.dram_tensor("w_g1", [HID, HID], f16, kind="ExternalInput")
    w_g2 = nc.dram_tensor("w_g2", [HID, OUT_CH], f16, kind="ExternalInput")
    b_gat_r = nc.dram_tensor("b_gat_r", [1, HC], dt, kind="ExternalInput")
    b_emb_r = nc.dram_tensor("b_emb_r", [1, HID], dt, kind="ExternalInput")
    b_g1_r = nc.dram_tensor("b_g1_r", [1, HID], dt, kind="ExternalInput")
    b_g2_r = nc.dram_tensor("b_g2_r", [1, OUT_CH], dt, kind="ExternalInput")
    out = nc.dram_tensor("out", [NODES_PER_CORE, OUT_CH], dt,
                         kind="ExternalOutput")

    gsem = nc.alloc_semaphore("gsem")
    gcount = [0]  # completed dma_gather calls so far

    RG = [list(range(NCORES))]

    def tiles_of(g):
        return min(GRP, Tmax - g * GRP)

    with tile.TileContext(nc) as tc:
        with (
            tc.tile_pool(name="const", bufs=1) as const,
            tc.tile_pool(name="dram", bufs=1, space="DRAM") as dram,
        ):
            # ---- DRAM scratch ----
            gat_table = dram.tile([NPAD, ROW], f16)
            h1T_loc = dram.tile([128, NODES_PER_CORE], f16)
            h1T_full = dram.tile([NCORES * 128, NODES_PER_CORE], f16)
            h2T_loc = dram.tile([128, NODES_PER_CORE], f16)
            h2T_full = dram.tile([NCORES * 128, NODES_PER_CORE], f16)

            # ---- constants in SBUF ----
            ident = const.tile([128, 128], f16)
            nc.sync.dma_start(out=ident[:], in_=ident_in[:])
            wgat_t = const.tile([IN_CH, HC], f16)
            nc.sync.dma_start(out=wgat_t[:], in_=w_gat[:])
            wad_t = const.tile([IN_CH, 16], f16)
            nc.sync.dma_start(out=wad_t[:], in_=wad[:])
            wd8_t = const.tile([IN_CH, HEADS], f16)
            nc.sync.dma_start(out=wd8_t[:], in_=wd8[:])
            wemb_t = const.tile([HC // 4, 4, HID], f16)   # [128, 4, 64] chunks
            nc.sync.dma_start(
                out=wemb_t[:],
                in_=w_emb[:].rearrange("(k p) f -> p k f", p=128),
            )
            wg1_t = const.tile([HID, HID], f16)
            nc.sync.dma_start(out=wg1_t[:], in_=w_g1[:])
            wg2_t = const.tile([HID, OUT_CH], f16)
            nc.sync.dma_start(out=wg2_t[:], in_=w_g2[:])
            # bias rows broadcast to 128 partitions (fp32)
            bgat_b = const.tile([128, HC], dt)
            r0 = const.tile([1, HC], dt, tag="r0")
            nc.sync.dma_start(out=r0[:], in_=b_gat_r[:])
            nc.gpsimd.partition_broadcast(bgat_b[:], r0[:1, :])
            bemb_b = const.tile([128, HID], dt)
            r1 = const.tile([1, HID], dt, tag="r1")
            nc.sync.dma_start(out=r1[:], in_=b_emb_r[:])
            nc.gpsimd.partition_broadcast(bemb_b[:], r1[:1, :])
            bg1_b = const.tile([128, HID], dt)
            r2 = const.tile([1, HID], dt, tag="r2")
            nc.sync.dma_start(out=r2[:], in_=b_g1_r[:])
            nc.gpsimd.partition_broadcast(bg1_b[:], r2[:1, :])
            bg2_b = const.tile([128, OUT_CH], dt)
            r3 = const.tile([1, OUT_CH], dt, tag="r3")
            nc.sync.dma_start(out=r3[:], in_=b_g2_r[:])
            nc.gpsimd.partition_broadcast(bg2_b[:], r3[:1, :])
            # iota row constant fp16: m_rows8[p, j] = j % 128
            m_rows8 = const.tile([128, GRP * 128], f16)
            r4 = const.tile([1, GRP * 128], f16, tag="r4")
            nc.sync.dma_start(out=r4[:], in_=iota_in[:])
            nc.gpsimd.partition_broadcast(m_rows8[:], r4[:1, :])
            # resident per-core metadata
            gidx_sb = const.tile([128, NB * Tmax * 8], mybir.dt.int16,
                                 tag="gi")
            nc.sync.dma_start(out=gidx_sb[:], in_=gidx[:])
            drt_all = const.tile([128, NB * Tmax], f16, tag="drt")
            nc.sync.dma_start(out=drt_all[:], in_=dstrel[:])
            # a_dst for own dst windows [128, NB*8] fp16
            adst_all = const.tile([128, NB * HEADS], f16, tag="adst")
            # shared SBUF copy of the allgathered transposed features
            hT_sb = const.tile([128, NCORES, NODES_PER_CORE], f16, tag="hT")
            # GCN h@W table, SBUF-resident
            tbl_sb = const.tile([128, NCHUNK, HID], f16, tag="tbl")

            def elu_inplace(pool, tile_ap, w, dtype):
                """tile_ap [128, w] <- elu(tile_ap); uses pool scratch."""
                xm = pool.tile([128, w], dtype, tag=f"elu{w}")
                nc.vector.tensor_scalar(out=xm[:], in0=tile_ap, scalar1=0.0,
                                        scalar2=None, op0=AO.min)
                nc.scalar.activation(out=xm[:], in_=xm[:], func=AF.Exp)
                nc.vector.tensor_scalar(out=tile_ap, in0=tile_ap, scalar1=0.0,
                                        scalar2=None, op0=AO.max)
                nc.vector.scalar_tensor_tensor(
                    out=tile_ap, in0=tile_ap, scalar=-1.0, in1=xm[:],
                    op0=AO.add, op1=AO.add)

            # =============================================================
            # Phase 0: build gat_table rows [a_src | xl | pad], a_dst windows
            # =============================================================
            with (
                tc.tile_pool(name="p0c", bufs=1) as p0c,
                tc.tile_pool(name="p0", bufs=3) as p0,
                tc.tile_pool(name="p0ps", bufs=2, space="PSUM") as p0ps,
            ):
                xT_sb = p0c.tile([128, NPAD], f16, tag="xT")
                nc.sync.dma_start(out=xT_sb[:], in_=xT[:])
                xdstT_sb = p0c.tile([128, NODES_PER_CORE], f16, tag="xdT")
                nc.sync.dma_start(out=xdstT_sb[:], in_=xdstT[:])
                for k in range(NCHUNK):
                    xl_ps = p0ps.tile([128, HC], dt, tag="xl", space="PSUM")
                    nc.tensor.matmul(out=xl_ps[:],
                                     lhsT=xT_sb[:, 128 * k:128 * (k + 1)],
                                     rhs=wgat_t[:], start=True, stop=True)
                    aw_ps = p0ps.tile([128, 16], dt, tag="aw", space="PSUM")
                    nc.tensor.matmul(out=aw_ps[:],
                                     lhsT=xT_sb[:, 128 * k:128 * (k + 1)],
                                     rhs=wad_t[:], start=True, stop=True)
                    row = p0.tile([128, 8 + HC], f16, tag="row")
                    nc.scalar.copy(out=row[:, 0:8], in_=aw_ps[:, 0:8])
                    nc.vector.tensor_copy(out=row[:, 8:8 + HC], in_=xl_ps[:])
                    nc.sync.dma_start(
                        out=gat_table[128 * k:128 * (k + 1), 0:8 + HC],
                        in_=row[:])
                # a_dst for own windows, from xdstT
                for b in range(NB):
                    ad_ps = p0ps.tile([128, HEADS], dt, tag="aw", space="PSUM")
                    nc.tensor.matmul(out=ad_ps[:],
                                     lhsT=xdstT_sb[:, 128 * b:128 * (b + 1)],
                                     rhs=wd8_t[:], start=True, stop=True)
                    nc.scalar.copy(
                        out=adst_all[:, b * HEADS:(b + 1) * HEADS],
                        in_=ad_ps[:])

            # =============================================================
            # gather: one call per dst block (all Tmax*128 edge rows).
            # The critical section [gather; wait] occupies gpsimd only, so
            # when it is emitted AFTER a block's compute, its descriptor
            # generation + DMA overlap that compute.
            # =============================================================
            def issue_gather(gpool, table, b, row_w, gtag):
                gtile = gpool.tile([128, Tmax, row_w], f16, tag=gtag)
                idx0 = b * Tmax * 8
                with tc.tile_critical(no_gpsimd_drain=True):
                    nc.gpsimd.dma_gather(
                        gtile[:], table[:],
                        gidx_sb[:, idx0:idx0 + Tmax * 8],
                        Tmax * 128, Tmax * 128, row_w,
                        single_packet=False,
                    ).then_inc(gsem, 16)
                    gcount[0] += 1
                    nc.gpsimd.wait_ge(gsem, 16 * gcount[0])
                return gtile

            # =============================================================
            # Phase 1: GAT blocks -> h1T_loc [128, 1280] fp16 (rows 0:64)
            # =============================================================
            with (
                tc.tile_pool(name="pg", bufs=2) as pg,       # gathered rows
                tc.tile_pool(name="pt", bufs=2) as pt,       # per-group scratch
                tc.tile_pool(name="pb", bufs=2) as pb,       # per-block scratch
                tc.tile_pool(name="ps_acc", bufs=2, space="PSUM") as ps_acc,
                tc.tile_pool(name="ps_ed", bufs=2, space="PSUM") as ps_ed,
                tc.tile_pool(name="ps_it", bufs=2, space="PSUM") as ps_it,
            ):
                gt_next = issue_gather(pg, gat_table, 0, ROW, "g")
                for b in range(NB):
                    gt = gt_next
                    msg_ps = ps_acc.tile([128, HC], dt, tag="msg",
                                         space="PSUM")
                    aux_ps = ps_acc.tile([128, 128], dt, tag="aux",
                                         space="PSUM")
                    adw = adst_all[:, b * HEADS:(b + 1) * HEADS]
                    for g in range(NGRP):
                        t = tiles_of(g)
                        t0 = g * GRP
                        gts = gt[:, t0:t0 + t, :]
                        # indicator I for the sub-group: [128, t*128] f16
                        I_g = pt.tile([128, GRP * 128], f16, tag="I")
                        nc.vector.tensor_tensor(
                            out=I_g[:, 0:t * 128].rearrange(
                                "p (t j) -> p t j", t=t),
                            in0=m_rows8[:, 0:t * 128].rearrange(
                                "p (t j) -> p t j", t=t),
                            in1=drt_all[:, b * Tmax + t0:b * Tmax + t0 + t]
                                .to_broadcast([128, t, 128]),
                            op=AO.is_equal)
                        # ed[e, th] = a_dst[dstrel_e, h] via transpose trick
                        ed_ps = ps_ed.tile([128, GRP * HEADS], dt, tag="ed",
                                           space="PSUM")
                        for tm in range(t):
                            it_ps = ps_it.tile([128, 128], f16, tag="it",
                                               space="PSUM")
                            nc.tensor.transpose(
                                out=it_ps[:],
                                in_=I_g[:, tm * 128:(tm + 1) * 128],
                                identity=ident[:])
                            IT = pt.tile([128, 128], f16, tag="IT")
                            nc.scalar.copy(out=IT[:], in_=it_ps[:])
                            nc.tensor.matmul(
                                out=ed_ps[:, tm * HEADS:(tm + 1) * HEADS],
                                lhsT=IT[:], rhs=adw, start=True, stop=True)
                        # el = leaky(a_src + ed); p = exp(el)
                        el = pt.tile([128, GRP * HEADS], f16, tag="el")
                        nc.vector.tensor_tensor(
                            out=el[:, 0:t * HEADS].rearrange(
                                "p (t h) -> p t h", t=t),
                            in0=gts[:, :, 0:8],
                            in1=ed_ps[:, 0:t * HEADS].rearrange(
                                "p (t h) -> p t h", t=t),
                            op=AO.add)
                        nc.vector.scalar_tensor_tensor(
                            out=el[:, 0:t * HEADS], in0=el[:, 0:t * HEADS],
                            scalar=NEG_SLOPE, in1=el[:, 0:t * HEADS],
                            op0=AO.mult, op1=AO.max)
                        p_blk = pt.tile([128, GRP, HEADS], f16, tag="p")
                        nc.scalar.activation(
                            out=p_blk[:, 0:t, :],
                            in_=el[:, 0:t * HEADS].rearrange(
                                "p (t h) -> p t h", t=t),
                            func=AF.Exp)
                        # broadcast p over channel dim on ACT, one big mult
                        p_brd = pt.tile([128, GRP, HEADS, HID], f16, tag="pb")
                        nc.scalar.copy(
                            out=p_brd[:, 0:t],
                            in_=p_blk[:, 0:t, :].to_broadcast(
                                [128, t, HEADS, HID]))
                        msg_g = pt.tile([128, GRP, HC], f16, tag="m")
                        nc.vector.tensor_tensor(
                            out=msg_g[:, 0:t].rearrange(
                                "p t (h c) -> p t h c", h=HEADS),
                            in0=gts[:, :, 8:8 + HC].rearrange(
                                "p t (h c) -> p t h c", h=HEADS),
                            in1=p_brd[:, 0:t],
                            op=AO.mult)
                        for tm in range(t):
                            tt = t0 + tm
                            nc.tensor.matmul(
                                out=msg_ps[:],
                                lhsT=I_g[:, tm * 128:(tm + 1) * 128],
                                rhs=msg_g[:, tm, :],
                                start=(tt == 0), stop=(tt == Tmax - 1))
                            nc.tensor.matmul(
                                out=aux_ps[:, 0:8],
                                lhsT=I_g[:, tm * 128:(tm + 1) * 128],
                                rhs=p_blk[:, tm, :],
                                start=(tt == 0), stop=(tt == Tmax - 1))
                    # ---- block epilogue ----
                    sinv = pb.tile([128, HEADS], dt, tag="sinv")
                    nc.vector.reciprocal(out=sinv[:], in_=aux_ps[:, 0:8])
                    o = pb.tile([128, HC], f16, tag="o")
                    nc.vector.tensor_tensor(
                        out=o[:].rearrange("p (h c) -> p h c", h=HEADS),
                        in0=msg_ps[:].rearrange("p (h c) -> p h c", h=HEADS),
                        in1=sinv[:].to_broadcast([128, HEADS, HID]),
                        op=AO.mult)
                    nc.vector.tensor_tensor(out=o[:], in0=o[:], in1=bgat_b[:],
                                            op=AO.add)
                    elu_inplace(pb, o[:], HC, f16)
                    # emb: h1 = elu(o @ w_emb + b_emb)
                    emb_ps = aux_ps[:, 64:128]
                    for kk in range(4):
                        ht_ps = ps_it.tile([128, 128], f16, tag="it",
                                           space="PSUM")
                        nc.tensor.transpose(out=ht_ps[:],
                                            in_=o[:, 128 * kk:128 * (kk + 1)],
                                            identity=ident[:])
                        ht = pb.tile([128, 128], f16, tag="ht")
                        nc.scalar.copy(out=ht[:], in_=ht_ps[:])
                        nc.tensor.matmul(out=emb_ps, lhsT=ht[:],
                                         rhs=wemb_t[:, kk, :],
                                         start=(kk == 0), stop=(kk == 3))
                    h1b = pb.tile([128, HID], f16, tag="h1b")
                    nc.vector.tensor_tensor(out=h1b[:], in0=emb_ps,
                                            in1=bemb_b[:], op=AO.add)
                    elu_inplace(pb, h1b[:], HID, f16)
                    # transpose; store to h1T_loc block cols
                    hd_ps = ps_it.tile([128, 128], f16, tag="it",
                                       space="PSUM")
                    nc.tensor.transpose(out=hd_ps[0:HID, :], in_=h1b[:],
                                        identity=ident[:])
                    hdT = pb.tile([HID, 128], f16, tag="hdT")
                    nc.scalar.copy(out=hdT[:], in_=hd_ps[0:HID, :])
                    nc.sync.dma_start(
                        out=h1T_loc[0:HID, 128 * b:128 * (b + 1)], in_=hdT[:])
                    # prefetch next block's gather AFTER this block's compute
                    # so descgen+DMA overlap it
                    if b + 1 < NB:
                        gt_next = issue_gather(pg, gat_table, b + 1, ROW, "g")

            # =============================================================
            # GCN layers: dense A^T blocks streamed from DRAM
            # =============================================================
            nc.gpsimd.collective_compute(
                "AllGather", mybir.AluOpType.bypass, replica_groups=RG,
                ins=[h1T_loc.opt()], outs=[h1T_full.opt()])

            def gcn_layer(hT_full, w_t, bias_b, h_out_loc, is_last):
                # table rows = h[n] @ W into SBUF (tbl_sb), from gathered hT
                nc.sync.dma_start(
                    out=hT_sb[:],
                    in_=hT_full[:].rearrange("(s p) f -> p s f", p=128))
                with (
                    tc.tile_pool(name="q0ps", bufs=2, space="PSUM") as q0ps,
                ):
                    for k in range(NCHUNK):
                        c, blk = divmod(k, NB)
                        t_ps = q0ps.tile([128, HID], dt, tag="t",
                                         space="PSUM")
                        nc.tensor.matmul(
                            out=t_ps[:],
                            lhsT=hT_sb[0:HID, c, 128 * blk:128 * (blk + 1)],
                            rhs=w_t[:], start=True, stop=True)
                        nc.scalar.copy(out=tbl_sb[:, k, :], in_=t_ps[:])
                # dense block loop
                with (
                    tc.tile_pool(name="qa", bufs=2) as qa,
                    tc.tile_pool(name="qb", bufs=2) as qb,
                    tc.tile_pool(name="qps", bufs=2, space="PSUM") as qps,
                ):
                    for b in range(NB):
                        a_sb = qa.tile([128, NCHUNK, 128], f16, tag="a")
                        nc.sync.dma_start(out=a_sb[:], in_=a_gcn[b])
                        acc_ps = qps.tile([128, HID], dt, tag="acc",
                                          space="PSUM")
                        for c in range(NCHUNK):
                            nc.tensor.matmul(
                                out=acc_ps[:],
                                lhsT=a_sb[:, c, :],
                                rhs=tbl_sb[:, c, :],
                                start=(c == 0), stop=(c == NCHUNK - 1))
                        # epilogue: out = acc + bias (then elu if mid layer)
                        if is_last:
                            ob = qb.tile([128, OUT_CH], dt, tag="obf")
                            nc.vector.tensor_tensor(
                                out=ob[:], in0=acc_ps[:], in1=bias_b,
                                op=AO.add)
                            nc.sync.dma_start(
                                out=h_out_loc[128 * b:128 * (b + 1)],
                                in_=ob[:])
                        else:
                            ob = qb.tile([128, HID], f16, tag="ob")
                            nc.vector.tensor_tensor(
                                out=ob[:], in0=acc_ps[:], in1=bias_b,
                                op=AO.add)
                            elu_inplace(qb, ob[:], HID, f16)
                            od_ps = qps.tile([128, 128], f16, tag="od",
                                             space="PSUM")
                            nc.tensor.transpose(out=od_ps[0:HID, :],
                                                in_=ob[:], identity=ident[:])
                            odT = qb.tile([HID, 128], f16, tag="odT")
                            nc.scalar.copy(out=odT[:], in_=od_ps[0:HID, :])
                            nc.sync.dma_start(
                                out=h_out_loc[0:HID, 128 * b:128 * (b + 1)],
                                in_=odT[:])

            gcn_layer(h1T_full, wg1_t, bg1_b[:], h2T_loc, False)
            nc.gpsimd.collective_compute(
                "AllGather", mybir.AluOpType.bypass, replica_groups=RG,
                ins=[h2T_loc.opt()], outs=[h2T_full.opt()])
            gcn_layer(h2T_full, wg2_t, bg2_b[:], out, True)

    nc.finalize()
    return nc


# ---------------------------------------------------------------------------
def _run(inputs, trace=False, **run_kw):
    from concourse import bass_utils

    x = np.asarray(inputs["x"], np.float32)
    edge_index = np.asarray(inputs["edge_index"])
    W_gat = np.asarray(inputs["W_gat"], np.float32)
    att_src = np.asarray(inputs["att_src"], np.float32)
    att_dst = np.asarray(inputs["att_dst"], np.float32)
    b_gat = np.asarray(inputs["b_gat"], np.float32)
    W_emb = np.asarray(inputs["W_emb"], np.float32)
    b_emb = np.asarray(inputs["b_emb"], np.float32)
    W_g1 = np.asarray(inputs["W_g1"], np.float32)
    b_g1 = np.asarray(inputs["b_g1"], np.float32)
    W_g2 = np.asarray(inputs["W_g2"], np.float32)
    b_g2 = np.asarray(inputs["b_g2"], np.float32)

    Tmax, NGRP, idx_host, dstrel_host, a_host = _host_prep(edge_index)
    nc = _build_nc(Tmax, NGRP)

    x_big = np.zeros((NCORES * NODES_PER_CORE, IN_CH), np.float32)
    x_big[:N] = x
    xT = x_big[:NPAD].T.astype(np.float16).copy()
    # Wa[k,h] = sum_c W_gat[k, h*HID+c] * att_src[h,c]  (same for Wd/att_dst)
    Wr = W_gat.reshape(IN_CH, HEADS, HID)
    Wa = np.einsum("khc,hc->kh", Wr, att_src).astype(np.float16)
    Wd = np.einsum("khc,hc->kh", Wr, att_dst).astype(np.float16)

    common = {
        "xT": xT,
        "ident_in": np.eye(128, dtype=np.float16),
        "iota_in": (np.arange(GRP * 128) % 128).astype(np.float16)[None, :],
        "w_gat": W_gat.astype(np.float16),
        "wad": np.concatenate([Wa, Wd], axis=1),
        "wd8": Wd,
        "w_emb": W_emb.astype(np.float16),
        "w_g1": W_g1.astype(np.float16),
        "w_g2": W_g2.astype(np.float16),
        "b_gat_r": b_gat[None, :], "b_emb_r": b_emb[None, :],
        "b_g1_r": b_g1[None, :], "b_g2_r": b_g2[None, :],
    }
    in_maps = []
    for c in range(NCORES):
        m = dict(common)
        m["xdstT"] = (
            x_big[c * NODES_PER_CORE:(c + 1) * NODES_PER_CORE].T
            .astype(np.float16).copy())
        m["gidx"] = idx_host[c]
        m["dstrel"] = dstrel_host[c]
        m["a_gcn"] = a_host[c]
        in_maps.append(m)

    res = bass_utils.run_bass_kernel_spmd(
        nc, in_maps, core_ids=list(range(NCORES)), trace=trace, **run_kw)
    full = np.concatenate([res.results[c]["out"] for c in range(NCORES)],
                          axis=0)
    return full[:N], res


def kernel(**inputs) -> np.ndarray:
    out, _ = _run(inputs, trace=False)
    return out


# revision 19
# speedup vs baseline: 2.1515x; 1.0530x over previous
"""GAT + 2x GCN message-passing model on 8 Trainium2 NeuronCores.

Sharding: nodes are partitioned across the 8 cores in aligned 1280-row
ranges (10 blocks of 128 dst nodes per core); every edge is owned by the
core that owns its destination node. Weights are replicated.

GAT layer: per-edge dma_gather of fp16 [a_src | xl] rows from a
replicated node table (one gather per dst block, prefetched one block
ahead so the SWDGE descriptor generation overlaps compute), attention
via batched fp16 DVE ops + indicator-matmul scatter.

GCN layers: fully dense — the per-edge weight dinv[src]*dinv[dst] is
host-known, so the host ships a blocked A^T matrix (fp16) and each layer
is a streamed dense matmul A_b @ (h @ W); no gathers at all. The h@W
table lives in SBUF. Between layers the per-core node shards are
exchanged with AllGather in a transposed fp16 layout.

Self-contained: hardcodes the problem shapes (N=10000, E=320000, IN=128,
HID=64, HEADS=8, OUT=64, neg_slope=0.2).
"""
import math

import numpy as np

# ---------------------------------------------------------------------------
# problem constants
N = 10000
E = 320000
IN_CH = 128
HID = 64
HEADS = 8
OUT_CH = 64
NEG_SLOPE = 0.2

NCORES = 8
NODES_PER_CORE = 1280          # 10 blocks of 128
NB = NODES_PER_CORE // 128     # dst blocks per core
NCHUNK = 79                    # ceil(10000/128) node chunks (rows 0..10111)
NPAD = NCHUNK * 128            # 10112 padded node-table rows
HC = HEADS * HID               # 512
ROW = 640                      # gat table row fp16: a_src(8)|xl(512)|pad(120)
GRP = 8                        # edge tiles (of 128) per compute sub-group


# ---------------------------------------------------------------------------
# Workaround for walrus codegen 'Too many sync wait commands' on the Tile
# kernel-tail Drain: spread the collected waits one-per-NoOp before the drain.
def _apply_tile_drain_patch():
    import concourse.mybir as mybir
    import concourse.tile as tile_mod
    from concourse.vector_clock import ScopedClock

    if getattr(tile_mod.TileContext, "_drain_patch_applied", False):
        return

    def _patched(self, tick_clock, wait_clock):
        nc = self.nc
        carrier = nc.sync.nop(nofuse=True)
        wait_clock.add_sem_waits(
            carrier.ins, ScopedClock({None: tick_clock.global_clock})
        )
        si = carrier.ins.sync_info
        if si is not None and si.on_wait and len(si.on_wait) > 1:
            waits = list(si.on_wait)
            si.on_wait = waits[:1]
            carrier.ins.sync_info = si
            for w in waits[1:]:
                extra = nc.sync.nop(nofuse=True)
                extra.ins.sync_info = mybir.SyncInfo(on_wait=[w], on_update=[])
        nc.sync.drain()
        nc.all_engine_barrier()
        assert self.sems is not None
        popped = nc._tile_sem_poison_stack.pop()
        assert popped is self._sem_poison
        nc.clear_and_free_semaphores(list(self.sems.allocated().values()))
        nc.all_engine_barrier()

    tile_mod.TileContext._drain_and_barrier = _patched
    tile_mod.TileContext._drain_patch_applied = True


# ---------------------------------------------------------------------------
def _wrap_idx(idx):
    """[n] -> [128, n//16] int16: idx i at [i % 16, i // 16], replicated x8."""
    w = idx.astype(np.int16).reshape(-1, 16).T
    return np.tile(w, (8, 1))


def _host_prep(edge_index):
    """Edge prep: GAT gather indices + dstrel, dense GCN A^T blocks.

    Returns (Tmax, NGRP, idx_host, dstrel_host, a_host) where
      idx_host:    [NCORES, 128, NB*Tmax*8] int16 gather indices
      dstrel_host: [NCORES, 128, NB*Tmax] fp16 (dst - block_base, pad -1000)
      a_host:      [NCORES, NCHUNK, 128, NB*128] fp16: a_host[c, k, j, d] =
                   A[dst=d, src=k*128+j] = mult * dinv[src] * dinv[dst]
    """
    src = np.concatenate([edge_index[0], np.arange(N, dtype=np.int64)])
    dst = np.concatenate([edge_index[1], np.arange(N, dtype=np.int64)])

    # degrees incl. self loop, from dst counts
    deg = np.bincount(dst, minlength=N).astype(np.float32)
    dinv = 1.0 / np.sqrt(deg)

    # dense A^T blocks: flat[dst_block, src_chunk, src_rel, dst_rel]
    a_flat = np.zeros(NCORES * NB * NCHUNK * 128 * 128, np.float32)
    db = dst // 128
    sc = src // 128
    sr = src % 128
    dr = dst % 128
    lin = ((db * NCHUNK + sc) * 128 + sr) * 128 + dr
    np.add.at(a_flat, lin, dinv[src] * dinv[dst])
    a_all = a_flat.reshape(NCORES, NB, NCHUNK, 128, 128).astype(np.float16)
    # rearrange to [core, chunk, j(src_rel), dst(b, dst_rel)] for rhs slices
    a_host = np.ascontiguousarray(a_all.transpose(0, 2, 3, 1, 4)).reshape(
        NCORES, NCHUNK, 128, NB * 128)

    order = np.argsort(dst, kind="stable")
    src_s = src[order]
    dst_s = dst[order]
    chunk = dst_s // 128
    counts = np.bincount(chunk, minlength=NCORES * NB)
    offsets = np.zeros(NCORES * NB + 1, np.int64)
    offsets[1:] = np.cumsum(counts)
    Tmax = max(1, math.ceil(counts.max() / 128))
    NGRP = math.ceil(Tmax / GRP)
    EPAD = Tmax * 128

    idx_host = np.zeros((NCORES, 128, NB * Tmax * 8), np.int16)
    dstrel_host = np.full((NCORES, 128, NB * Tmax), -1000.0, np.float16)
    for c in range(NCORES):
        for b in range(NB):
            g = c * NB + b
            lo, hi = offsets[g], offsets[g + 1]
            s = np.zeros(EPAD, np.int64)          # pad src = 0 (harmless row)
            s[: hi - lo] = src_s[lo:hi]
            r = np.full(Tmax * 128, -1000.0, np.float32)
            r[: hi - lo] = (dst_s[lo:hi] - g * 128).astype(np.float32)
            idx_host[c, :, b * Tmax * 8:(b + 1) * Tmax * 8] = _wrap_idx(s)
            dstrel_host[c, :, b * Tmax:(b + 1) * Tmax] = \
                r.reshape(Tmax, 128).T.astype(np.float16)
    return Tmax, NGRP, idx_host, dstrel_host, a_host


# ---------------------------------------------------------------------------
def _build_nc(Tmax, NGRP):
    import concourse.bacc as bacc
    import concourse.mybir as mybir
    import concourse.tile as tile

    _apply_tile_drain_patch()
    AO = mybir.AluOpType
    AF = mybir.ActivationFunctionType
    dt = mybir.dt.float32
    f16 = mybir.dt.float16

    nc = bacc.Bacc("TRN2")

    # ---- inputs ----
    xT = nc.dram_tensor("xT", [128, NPAD], f16, kind="ExternalInput")
    xdstT = nc.dram_tensor("xdstT", [128, NODES_PER_CORE], f16,
                           kind="ExternalInput")
    ident_in = nc.dram_tensor("ident_in", [128, 128], f16, kind="ExternalInput")
    iota_in = nc.dram_tensor("iota_in", [1, GRP * 128], f16,
                             kind="ExternalInput")
    gidx = nc.dram_tensor("gidx", [128, NB * Tmax * 8], mybir.dt.int16,
                          kind="ExternalInput")
    dstrel = nc.dram_tensor("dstrel", [128, NB * Tmax], f16,
                            kind="ExternalInput")
    a_gcn = nc.dram_tensor("a_gcn", [NCHUNK, 128, NB * 128], f16,
                           kind="ExternalInput")
    b_g1_c = nc.dram_tensor("b_g1_c", [HID, 1], dt, kind="ExternalInput")
    b_g2_c = nc.dram_tensor("b_g2_c", [OUT_CH, 1], dt, kind="ExternalInput")
    w_gat = nc.dram_tensor("w_gat", [IN_CH, HC], f16, kind="ExternalInput")
    wad = nc.dram_tensor("wad", [IN_CH, 16], f16, kind="ExternalInput")
    wd8 = nc.dram_tensor("wd8", [IN_CH, HEADS], f16, kind="ExternalInput")
    w_emb = nc.dram_tensor("w_emb", [HC, HID], f16, kind="ExternalInput")
    w_g1 = nc.dram_tensor("w_g1", [HID, HID], f16, kind="ExternalInput")
    w_g2 = nc.dram_tensor("w_g2", [HID, OUT_CH], f16, kind="ExternalInput")
    b_gat_r = nc.dram_tensor("b_gat_r", [1, HC], dt, kind="ExternalInput")
    b_emb_r = nc.dram_tensor("b_emb_r", [1, HID], dt, kind="ExternalInput")
    out = nc.dram_tensor("out", [NODES_PER_CORE, OUT_CH], dt,
                         kind="ExternalOutput")

    gsem = nc.alloc_semaphore("gsem")
    gcount = [0]  # completed dma_gather calls so far

    RG = [list(range(NCORES))]

    def tiles_of(g):
        return min(GRP, Tmax - g * GRP)

    with tile.TileContext(nc) as tc:
        with (
            tc.tile_pool(name="const", bufs=1) as const,
            tc.tile_pool(name="dram", bufs=1, space="DRAM") as dram,
        ):
            # ---- DRAM scratch ----
            gat_table = dram.tile([NPAD, ROW], f16)
            h1T_loc = dram.tile([128, NODES_PER_CORE], f16)
            h1T_full = dram.tile([NCORES * 128, NODES_PER_CORE], f16)
            h2T_loc = dram.tile([128, NODES_PER_CORE], f16)
            h2T_full = dram.tile([NCORES * 128, NODES_PER_CORE], f16)

            # ---- constants in SBUF ----
            ident = const.tile([128, 128], f16)
            nc.sync.dma_start(out=ident[:], in_=ident_in[:])
            wgat_t = const.tile([IN_CH, HC], f16)
            nc.sync.dma_start(out=wgat_t[:], in_=w_gat[:])
            wad_t = const.tile([IN_CH, 16], f16)
            nc.sync.dma_start(out=wad_t[:], in_=wad[:])
            wd8_t = const.tile([IN_CH, HEADS], f16)
            nc.sync.dma_start(out=wd8_t[:], in_=wd8[:])
            wemb_t = const.tile([HC // 4, 4, HID], f16)   # [128, 4, 64] chunks
            nc.sync.dma_start(
                out=wemb_t[:],
                in_=w_emb[:].rearrange("(k p) f -> p k f", p=128),
            )
            wg1_t = const.tile([HID, HID], f16)
            nc.sync.dma_start(out=wg1_t[:], in_=w_g1[:])
            wg2_t = const.tile([HID, OUT_CH], f16)
            nc.sync.dma_start(out=wg2_t[:], in_=w_g2[:])
            # bias rows broadcast to 128 partitions (fp32)
            bgat_b = const.tile([128, HC], dt)
            r0 = const.tile([1, HC], dt, tag="r0")
            nc.sync.dma_start(out=r0[:], in_=b_gat_r[:])
            nc.gpsimd.partition_broadcast(bgat_b[:], r0[:1, :])
            bemb_b = const.tile([128, HID], dt)
            r1 = const.tile([1, HID], dt, tag="r1")
            nc.sync.dma_start(out=r1[:], in_=b_emb_r[:])
            nc.gpsimd.partition_broadcast(bemb_b[:], r1[:1, :])
            bg1_c = const.tile([HID, 1], dt, tag="bg1c")
            nc.sync.dma_start(out=bg1_c[:], in_=b_g1_c[:])
            bg2_c = const.tile([OUT_CH, 1], dt, tag="bg2c")
            nc.sync.dma_start(out=bg2_c[:], in_=b_g2_c[:])
            # iota row constant fp16: m_rows8[p, j] = j % 128
            m_rows8 = const.tile([128, GRP * 128], f16)
            r4 = const.tile([1, GRP * 128], f16, tag="r4")
            nc.sync.dma_start(out=r4[:], in_=iota_in[:])
            nc.gpsimd.partition_broadcast(m_rows8[:], r4[:1, :])
            # resident per-core metadata
            gidx_sb = const.tile([128, NB * Tmax * 8], mybir.dt.int16,
                                 tag="gi")
            nc.sync.dma_start(out=gidx_sb[:], in_=gidx[:])
            drt_all = const.tile([128, NB * Tmax], f16, tag="drt")
            nc.sync.dma_start(out=drt_all[:], in_=dstrel[:])
            # a_dst for own dst windows [128, NB*8] fp16
            adst_all = const.tile([128, NB * HEADS], f16, tag="adst")
            # shared SBUF copy of the allgathered transposed features
            hT_sb = const.tile([128, NCORES, NODES_PER_CORE], f16, tag="hT")
            # GCN h@W table, SBUF-resident
            tbl_sb = const.tile([128, NCHUNK, HID], f16, tag="tbl")

            def elu_inplace(pool, tile_ap, w, dtype, pdim=128):
                """tile_ap [pdim, w] <- elu(tile_ap); uses pool scratch."""
                xm = pool.tile([pdim, w], dtype, tag=f"elu{w}")
                nc.vector.tensor_scalar(out=xm[:], in0=tile_ap, scalar1=0.0,
                                        scalar2=None, op0=AO.min)
                nc.scalar.activation(out=xm[:], in_=xm[:], func=AF.Exp)
                nc.vector.tensor_scalar(out=tile_ap, in0=tile_ap, scalar1=0.0,
                                        scalar2=None, op0=AO.max)
                nc.vector.scalar_tensor_tensor(
                    out=tile_ap, in0=tile_ap, scalar=-1.0, in1=xm[:],
                    op0=AO.add, op1=AO.add)

            # =============================================================
            # Phase 0: build gat_table rows [a_src | xl | pad], a_dst windows
            # =============================================================
            with (
                tc.tile_pool(name="p0c", bufs=1) as p0c,
                tc.tile_pool(name="p0", bufs=3) as p0,
                tc.tile_pool(name="p0ps", bufs=2, space="PSUM") as p0ps,
            ):
                xT_sb = p0c.tile([128, NPAD], f16, tag="xT")
                nc.sync.dma_start(out=xT_sb[:], in_=xT[:])
                xdstT_sb = p0c.tile([128, NODES_PER_CORE], f16, tag="xdT")
                nc.sync.dma_start(out=xdstT_sb[:], in_=xdstT[:])
                for k in range(NCHUNK):
                    xl_ps = p0ps.tile([128, HC], dt, tag="xl", space="PSUM")
                    nc.tensor.matmul(out=xl_ps[:],
                                     lhsT=xT_sb[:, 128 * k:128 * (k + 1)],
                                     rhs=wgat_t[:], start=True, stop=True)
                    aw_ps = p0ps.tile([128, 16], dt, tag="aw", space="PSUM")
                    nc.tensor.matmul(out=aw_ps[:],
                                     lhsT=xT_sb[:, 128 * k:128 * (k + 1)],
                                     rhs=wad_t[:], start=True, stop=True)
                    row = p0.tile([128, 8 + HC], f16, tag="row")
                    nc.scalar.copy(out=row[:, 0:8], in_=aw_ps[:, 0:8])
                    nc.vector.tensor_copy(out=row[:, 8:8 + 256],
                                          in_=xl_ps[:, 0:256])
                    nc.scalar.copy(out=row[:, 8 + 256:8 + HC],
                                   in_=xl_ps[:, 256:HC])
                    nc.sync.dma_start(
                        out=gat_table[128 * k:128 * (k + 1), 0:8 + HC],
                        in_=row[:])
                # a_dst for own windows, from xdstT
                for b in range(NB):
                    ad_ps = p0ps.tile([128, HEADS], dt, tag="aw", space="PSUM")
                    nc.tensor.matmul(out=ad_ps[:],
                                     lhsT=xdstT_sb[:, 128 * b:128 * (b + 1)],
                                     rhs=wd8_t[:], start=True, stop=True)
                    nc.scalar.copy(
                        out=adst_all[:, b * HEADS:(b + 1) * HEADS],
                        in_=ad_ps[:])

            # =============================================================
            # gather: one call per dst block (all Tmax*128 edge rows).
            # The critical section [gather; wait] occupies gpsimd only, so
            # when it is emitted AFTER a block's compute, its descriptor
            # generation + DMA overlap that compute.
            # =============================================================
            def issue_gather(gpool, table, b, row_w, gtag, defer=True):
                gtile = gpool.tile([128, Tmax, row_w], f16, tag=gtag)
                idx0 = b * Tmax * 8
                with tc.tile_critical(no_gpsimd_drain=True):
                    nc.gpsimd.dma_gather(
                        gtile[:], table[:],
                        gidx_sb[:, idx0:idx0 + Tmax * 8],
                        Tmax * 128, Tmax * 128, row_w,
                        single_packet=False,
                    ).then_inc(gsem, 16)
                    if defer:
                        # run descgen+DMA concurrently with previously
                        # emitted compute; ordering vs the buffer's prior
                        # readers is guaranteed transitively by the
                        # previous section's marker (Pool FIFO).
                        tc.wait_critical_data_deps()
                    gcount[0] += 1
                    nc.gpsimd.wait_ge(gsem, 16 * gcount[0])
                return gtile

            # =============================================================
            # Phase 1: GAT blocks -> h1T_loc [128, 1280] fp16 (rows 0:64)
            # =============================================================
            with (
                tc.tile_pool(name="pg", bufs=2) as pg,       # gathered rows
                tc.tile_pool(name="pt", bufs=2) as pt,       # per-group scratch
                tc.tile_pool(name="pb", bufs=2) as pb,       # per-block scratch
                tc.tile_pool(name="ps_acc", bufs=2, space="PSUM") as ps_acc,
                tc.tile_pool(name="ps_ed", bufs=2, space="PSUM") as ps_ed,
                tc.tile_pool(name="ps_it", bufs=2, space="PSUM") as ps_it,
            ):
                gt_next = issue_gather(pg, gat_table, 0, ROW, "g", defer=False)
                for b in range(NB):
                    gt = gt_next
                    msg_ps = ps_acc.tile([128, HC], dt, tag="msg",
                                         space="PSUM")
                    aux_ps = ps_acc.tile([128, 128], dt, tag="aux",
                                         space="PSUM")
                    adw = adst_all[:, b * HEADS:(b + 1) * HEADS]
                    for g in range(NGRP):
                        t = tiles_of(g)
                        t0 = g * GRP
                        gts = gt[:, t0:t0 + t, :]
                        # indicator I for the sub-group: [128, t*128] f16
                        I_g = pt.tile([128, GRP * 128], f16, tag="I")
                        nc.vector.tensor_tensor(
                            out=I_g[:, 0:t * 128].rearrange(
                                "p (t j) -> p t j", t=t),
                            in0=m_rows8[:, 0:t * 128].rearrange(
                                "p (t j) -> p t j", t=t),
                            in1=drt_all[:, b * Tmax + t0:b * Tmax + t0 + t]
                                .to_broadcast([128, t, 128]),
                            op=AO.is_equal)
                        # ed[e, th] = a_dst[dstrel_e, h] via transpose trick
                        ed_ps = ps_ed.tile([128, GRP * HEADS], dt, tag="ed",
                                           space="PSUM")
                        for tm in range(t):
                            it_ps = ps_it.tile([128, 128], f16, tag="it",
                                               space="PSUM")
                            nc.tensor.transpose(
                                out=it_ps[:],
                                in_=I_g[:, tm * 128:(tm + 1) * 128],
                                identity=ident[:])
                            IT = pt.tile([128, 128], f16, tag="IT")
                            nc.scalar.copy(out=IT[:], in_=it_ps[:])
                            nc.tensor.matmul(
                                out=ed_ps[:, tm * HEADS:(tm + 1) * HEADS],
                                lhsT=IT[:], rhs=adw, start=True, stop=True)
                        # el = leaky(a_src + ed); p = exp(el)
                        el = pt.tile([128, GRP * HEADS], f16, tag="el")
                        nc.vector.tensor_tensor(
                            out=el[:, 0:t * HEADS].rearrange(
                                "p (t h) -> p t h", t=t),
                            in0=gts[:, :, 0:8],
                            in1=ed_ps[:, 0:t * HEADS].rearrange(
                                "p (t h) -> p t h", t=t),
                            op=AO.add)
                        nc.vector.scalar_tensor_tensor(
                            out=el[:, 0:t * HEADS], in0=el[:, 0:t * HEADS],
                            scalar=NEG_SLOPE, in1=el[:, 0:t * HEADS],
                            op0=AO.mult, op1=AO.max)
                        p_blk = pt.tile([128, GRP, HEADS], f16, tag="p")
                        nc.scalar.activation(
                            out=p_blk[:, 0:t, :],
                            in_=el[:, 0:t * HEADS].rearrange(
                                "p (t h) -> p t h", t=t),
                            func=AF.Exp)
                        # broadcast p over channel dim on ACT, one big mult
                        p_brd = pt.tile([128, GRP, HEADS, HID], f16, tag="pb")
                        nc.scalar.copy(
                            out=p_brd[:, 0:t],
                            in_=p_blk[:, 0:t, :].to_broadcast(
                                [128, t, HEADS, HID]))
                        msg_g = pt.tile([128, GRP, HC], f16, tag="m")
                        nc.vector.tensor_tensor(
                            out=msg_g[:, 0:t].rearrange(
                                "p t (h c) -> p t h c", h=HEADS),
                            in0=gts[:, :, 8:8 + HC].rearrange(
                                "p t (h c) -> p t h c", h=HEADS),
                            in1=p_brd[:, 0:t],
                            op=AO.mult)
                        for tm in range(t):
                            tt = t0 + tm
                            nc.tensor.matmul(
                                out=msg_ps[:],
                                lhsT=I_g[:, tm * 128:(tm + 1) * 128],
                                rhs=msg_g[:, tm, :],
                                start=(tt == 0), stop=(tt == Tmax - 1))
                            nc.tensor.matmul(
                                out=aux_ps[:, 0:8],
                                lhsT=I_g[:, tm * 128:(tm + 1) * 128],
                                rhs=p_blk[:, tm, :],
                                start=(tt == 0), stop=(tt == Tmax - 1))
                    # ---- block epilogue ----
                    sinv = pb.tile([128, HEADS], dt, tag="sinv")
                    nc.vector.tensor_scalar(out=sinv[:], in0=aux_ps[:, 0:8],
                                            scalar1=1e-30, scalar2=None,
                                            op0=AO.max)
                    nc.vector.reciprocal(out=sinv[:], in_=sinv[:])
                    o = pb.tile([128, HC], f16, tag="o")
                    nc.vector.tensor_tensor(
                        out=o[:].rearrange("p (h c) -> p h c", h=HEADS),
                        in0=msg_ps[:].rearrange("p (h c) -> p h c", h=HEADS),
                        in1=sinv[:].to_broadcast([128, HEADS, HID]),
                        op=AO.mult)
                    nc.vector.tensor_tensor(out=o[:], in0=o[:], in1=bgat_b[:],
                                            op=AO.add)
                    elu_inplace(pb, o[:], HC, f16)
                    # emb: h1 = elu(o @ w_emb + b_emb)
                    emb_ps = aux_ps[:, 64:128]
                    for kk in range(4):
                        ht_ps = ps_it.tile([128, 128], f16, tag="it",
                                           space="PSUM")
                        nc.tensor.transpose(out=ht_ps[:],
                                            in_=o[:, 128 * kk:128 * (kk + 1)],
                                            identity=ident[:])
                        ht = pb.tile([128, 128], f16, tag="ht")
                        nc.scalar.copy(out=ht[:], in_=ht_ps[:])
                        nc.tensor.matmul(out=emb_ps, lhsT=ht[:],
                                         rhs=wemb_t[:, kk, :],
                                         start=(kk == 0), stop=(kk == 3))
                    h1b = pb.tile([128, HID], f16, tag="h1b")
                    nc.vector.tensor_tensor(out=h1b[:], in0=emb_ps,
                                            in1=bemb_b[:], op=AO.add)
                    elu_inplace(pb, h1b[:], HID, f16)
                    # transpose; store to h1T_loc block cols
                    hd_ps = ps_it.tile([128, 128], f16, tag="it",
                                       space="PSUM")
                    nc.tensor.transpose(out=hd_ps[0:HID, :], in_=h1b[:],
                                        identity=ident[:])
                    hdT = pb.tile([HID, 128], f16, tag="hdT")
                    nc.scalar.copy(out=hdT[:], in_=hd_ps[0:HID, :])
                    nc.sync.dma_start(
                        out=h1T_loc[0:HID, 128 * b:128 * (b + 1)], in_=hdT[:])
                    # prefetch next block's gather AFTER this block's compute
                    # so descgen+DMA overlap it
                    if b + 1 < NB:
                        gt_next = issue_gather(pg, gat_table, b + 1, ROW, "g")

            # =============================================================
            # GCN layers: dense A^T blocks streamed from DRAM
            # =============================================================
            nc.gpsimd.collective_compute(
                "AllGather", mybir.AluOpType.bypass, replica_groups=RG,
                ins=[h1T_loc.opt()], outs=[h1T_full.opt()])

            def gcn_layer(hT_full, w_t, bias_col, h_out_loc, is_last):
                # table rows = h[n] @ W into SBUF (tbl_sb), from gathered hT
                nc.sync.dma_start(
                    out=hT_sb[:],
                    in_=hT_full[:].rearrange("(s p) f -> p s f", p=128))
                with (
                    tc.tile_pool(name="q0ps", bufs=2, space="PSUM") as q0ps,
                ):
                    for k in range(NCHUNK):
                        c, blk = divmod(k, NB)
                        t_ps = q0ps.tile([128, HID], dt, tag="t",
                                         space="PSUM")
                        nc.tensor.matmul(
                            out=t_ps[:],
                            lhsT=hT_sb[0:HID, c, 128 * blk:128 * (blk + 1)],
                            rhs=w_t[:], start=True, stop=True)
                        nc.scalar.copy(out=tbl_sb[:, k, :], in_=t_ps[:])
                # dense accumulation over all chunks, transposed output
                # accT [HID, 1280] = sum_c tbl_c^T @ A^T_c
                ND = NB * 128
                with (
                    tc.tile_pool(name="qa", bufs=3) as qa,
                    tc.tile_pool(name="qb", bufs=1) as qb,
                    tc.tile_pool(name="qps", bufs=1, space="PSUM") as qps,
                    tc.tile_pool(name="qps2", bufs=2, space="PSUM") as qps2,
                ):
                    spans = [(0, 512), (512, 1024), (1024, ND)]
                    accs = []
                    for si, (lo, hi) in enumerate(spans):
                        acc = qps.tile([HID, hi - lo], dt, tag=f"acc{si}",
                                       name=f"acc{si}", space="PSUM")
                        accs.append(acc)
                    for c in range(NCHUNK):
                        a_sb = qa.tile([128, ND], f16, tag="a")
                        nc.sync.dma_start(out=a_sb[:], in_=a_gcn[c])
                        for si, (lo, hi) in enumerate(spans):
                            nc.tensor.matmul(
                                out=accs[si][:], lhsT=tbl_sb[:, c, :],
                                rhs=a_sb[:, lo:hi],
                                start=(c == 0), stop=(c == NCHUNK - 1))
                    # epilogue in transposed layout [HID, 1280]
                    obT = qb.tile([HID, ND], f16, tag="obT")
                    for si, (lo, hi) in enumerate(spans):
                        nc.vector.tensor_scalar(
                            out=obT[:, lo:hi], in0=accs[si][:],
                            scalar1=bias_col, scalar2=None, op0=AO.add)
                    if is_last:
                        # transpose back per block, write node-major f32
                        for b in range(NB):
                            ob_ps = qps2.tile([128, OUT_CH], f16, tag="obp",
                                              space="PSUM")
                            nc.tensor.transpose(
                                out=ob_ps[:],
                                in_=obT[:, 128 * b:128 * (b + 1)],
                                identity=ident[0:OUT_CH, 0:OUT_CH])
                            obf = qb.tile([128, OUT_CH], dt, tag="obf")
                            nc.scalar.copy(out=obf[:], in_=ob_ps[:])
                            nc.sync.dma_start(
                                out=h_out_loc[128 * b:128 * (b + 1)],
                                in_=obf[:])
                    else:
                        elu_inplace(qb, obT[:], ND, f16, pdim=HID)
                        nc.sync.dma_start(
                            out=h_out_loc[0:HID, :], in_=obT[:])

            gcn_layer(h1T_full, wg1_t, bg1_c[:], h2T_loc, False)
            nc.gpsimd.collective_compute(
                "AllGather", mybir.AluOpType.bypass, replica_groups=RG,
                ins=[h2T_loc.opt()], outs=[h2T_full.opt()])
            gcn_layer(h2T_full, wg2_t, bg2_c[:], out, True)

    nc.finalize()
    return nc


# ---------------------------------------------------------------------------
def _run(inputs, trace=False, **run_kw):
    from concourse import bass_utils

    x = np.asarray(inputs["x"], np.float32)
    edge_index = np.asarray(inputs["edge_index"])
    W_gat = np.asarray(inputs["W_gat"], np.float32)
    att_src = np.asarray(inputs["att_src"], np.float32)
    att_dst = np.asarray(inputs["att_dst"], np.float32)
    b_gat = np.asarray(inputs["b_gat"], np.float32)
    W_emb = np.asarray(inputs["W_emb"], np.float32)
    b_emb = np.asarray(inputs["b_emb"], np.float32)
    W_g1 = np.asarray(inputs["W_g1"], np.float32)
    b_g1 = np.asarray(inputs["b_g1"], np.float32)
    W_g2 = np.asarray(inputs["W_g2"], np.float32)
    b_g2 = np.asarray(inputs["b_g2"], np.float32)

    Tmax, NGRP, idx_host, dstrel_host, a_host = _host_prep(edge_index)
    nc = _build_nc(Tmax, NGRP)

    x_big = np.zeros((NCORES * NODES_PER_CORE, IN_CH), np.float32)
    x_big[:N] = x
    xT = x_big[:NPAD].T.astype(np.float16).copy()
    # Wa[k,h] = sum_c W_gat[k, h*HID+c] * att_src[h,c]  (same for Wd/att_dst)
    Wr = W_gat.reshape(IN_CH, HEADS, HID)
    Wa = np.einsum("khc,hc->kh", Wr, att_src).astype(np.float16)
    Wd = np.einsum("khc,hc->kh", Wr, att_dst).astype(np.float16)

    common = {
        "xT": xT,
        "ident_in": np.eye(128, dtype=np.float16),
        "iota_in": (np.arange(GRP * 128) % 128).astype(np.float16)[None, :],
        "w_gat": W_gat.astype(np.float16),
        "wad": np.concatenate([Wa, Wd], axis=1),
        "wd8": Wd,
        "w_emb": W_emb.astype(np.float16),
        "w_g1": W_g1.astype(np.float16),
        "w_g2": W_g2.astype(np.float16),
        "b_gat_r": b_gat[None, :], "b_emb_r": b_emb[None, :],
        "b_g1_r": b_g1[None, :], "b_g2_r": b_g2[None, :],
        "b_g1_c": b_g1[:, None].copy(), "b_g2_c": b_g2[:, None].copy(),
    }
    in_maps = []
    for c in range(NCORES):
        m = dict(common)
        m["xdstT"] = (
            x_big[c * NODES_PER_CORE:(c + 1) * NODES_PER_CORE].T
            .astype(np.float16).copy())
        m["gidx"] = idx_host[c]
        m["dstrel"] = dstrel_host[c]
        m["a_gcn"] = a_host[c]
        in_maps.append(m)

    res = bass_utils.run_bass_kernel_spmd(
        nc, in_maps, core_ids=list(range(NCORES)), trace=trace, **run_kw)
    full = np.concatenate([res.results[c]["out"] for c in range(NCORES)],
                          axis=0)
    return full[:N], res


def kernel(**inputs) -> np.ndarray:
    out, _ = _run(inputs, trace=False)
    return out


# revision 21
# speedup vs baseline: 2.4581x; 1.1425x over previous
"""GAT + 2x GCN message-passing model on 8 Trainium2 NeuronCores.

Sharding: nodes are partitioned across the 8 cores in aligned 1280-row
ranges (10 blocks of 128 dst nodes per core); every edge is owned by the
core that owns its destination node. Weights are replicated.

GAT layer: per-edge dma_gather of fp16 [a_src | xl] rows from a
replicated node table (one gather per dst block, prefetched one block
ahead so the SWDGE descriptor generation overlaps compute), attention
via batched fp16 DVE ops + indicator-matmul scatter.

GCN layers: fully dense — the per-edge weight dinv[src]*dinv[dst] is
host-known, so the host ships a blocked A^T matrix (fp16) and each layer
is a streamed dense matmul A_b @ (h @ W); no gathers at all. The h@W
table lives in SBUF. Between layers the per-core node shards are
exchanged with AllGather in a transposed fp16 layout.

Self-contained: hardcodes the problem shapes (N=10000, E=320000, IN=128,
HID=64, HEADS=8, OUT=64, neg_slope=0.2).
"""
import math

import numpy as np

# ---------------------------------------------------------------------------
# problem constants
N = 10000
E = 320000
IN_CH = 128
HID = 64
HEADS = 8
OUT_CH = 64
NEG_SLOPE = 0.2

NCORES = 8
NODES_PER_CORE = 1280          # 10 blocks of 128
NB = NODES_PER_CORE // 128     # dst blocks per core
NCHUNK = 79                    # ceil(10000/128) node chunks (rows 0..10111)
NPAD = NCHUNK * 128            # 10112 padded node-table rows
HC = HEADS * HID               # 512
ROW = 640                      # gat table row fp16: a_src(8)|xl(512)|pad(120)
GRP = 8                        # edge tiles (of 128) per compute sub-group


# ---------------------------------------------------------------------------
# Workaround for walrus codegen 'Too many sync wait commands' on the Tile
# kernel-tail Drain: spread the collected waits one-per-NoOp before the drain.
def _apply_tile_drain_patch():
    import concourse.mybir as mybir
    import concourse.tile as tile_mod
    from concourse.vector_clock import ScopedClock

    if getattr(tile_mod.TileContext, "_drain_patch_applied", False):
        return

    def _patched(self, tick_clock, wait_clock):
        nc = self.nc
        carrier = nc.sync.nop(nofuse=True)
        wait_clock.add_sem_waits(
            carrier.ins, ScopedClock({None: tick_clock.global_clock})
        )
        si = carrier.ins.sync_info
        if si is not None and si.on_wait and len(si.on_wait) > 1:
            waits = list(si.on_wait)
            si.on_wait = waits[:1]
            carrier.ins.sync_info = si
            for w in waits[1:]:
                extra = nc.sync.nop(nofuse=True)
                extra.ins.sync_info = mybir.SyncInfo(on_wait=[w], on_update=[])
        nc.sync.drain()
        nc.all_engine_barrier()
        assert self.sems is not None
        popped = nc._tile_sem_poison_stack.pop()
        assert popped is self._sem_poison
        nc.clear_and_free_semaphores(list(self.sems.allocated().values()))
        nc.all_engine_barrier()

    tile_mod.TileContext._drain_and_barrier = _patched
    tile_mod.TileContext._drain_patch_applied = True


# ---------------------------------------------------------------------------
def _wrap_idx(idx):
    """[n] -> [128, n//16] int16: idx i at [i % 16, i // 16], replicated x8."""
    w = idx.astype(np.int16).reshape(-1, 16).T
    return np.tile(w, (8, 1))


def _host_prep(edge_index):
    """Edge prep: GAT gather indices + dstrel, dense GCN A^T blocks.

    Returns (Tmax, NGRP, idx_host, dstrel_host, a_host) where
      idx_host:    [NCORES, 128, NB*Tmax*8] int16 gather indices
      dstrel_host: [NCORES, 128, NB*Tmax] fp16 (dst - block_base, pad -1000)
      a_host:      [NCORES, NCHUNK, 128, NB*128] fp16: a_host[c, k, j, d] =
                   A[dst=d, src=k*128+j] = mult * dinv[src] * dinv[dst]
    """
    src = np.concatenate([edge_index[0], np.arange(N, dtype=np.int64)])
    dst = np.concatenate([edge_index[1], np.arange(N, dtype=np.int64)])

    # degrees incl. self loop, from dst counts
    deg = np.bincount(dst, minlength=N).astype(np.float32)
    dinv = 1.0 / np.sqrt(deg)

    # dense A^T blocks: flat[dst_block, src_chunk, src_rel, dst_rel]
    a_flat = np.zeros(NCORES * NB * NCHUNK * 128 * 128, np.float32)
    db = dst // 128
    sc = src // 128
    sr = src % 128
    dr = dst % 128
    lin = ((db * NCHUNK + sc) * 128 + sr) * 128 + dr
    np.add.at(a_flat, lin, dinv[src] * dinv[dst])
    a_all = a_flat.reshape(NCORES, NB, NCHUNK, 128, 128).astype(np.float16)
    # rearrange to [core, chunk, j(src_rel), dst(b, dst_rel)] for rhs slices
    a_host = np.ascontiguousarray(a_all.transpose(0, 2, 3, 1, 4)).reshape(
        NCORES, NCHUNK, 128, NB * 128)

    order = np.argsort(dst, kind="stable")
    src_s = src[order]
    dst_s = dst[order]
    chunk = dst_s // 128
    counts = np.bincount(chunk, minlength=NCORES * NB)
    offsets = np.zeros(NCORES * NB + 1, np.int64)
    offsets[1:] = np.cumsum(counts)
    Tmax = max(1, math.ceil(counts.max() / 128))
    NGRP = math.ceil(Tmax / GRP)
    EPAD = Tmax * 128

    idx_host = np.zeros((NCORES, 128, NB * Tmax * 8), np.int16)
    # one-hot dst indicators per edge tile, host-built:
    #   i_host[c, b, e, t*128+j]  = 1 iff dst_rel(edge t*128+e of block b) == j
    #   it_host[c, b, j, t*128+e] = same, transposed within each tile
    i_host = np.zeros((NCORES, NB, 128, Tmax * 128), np.float16)
    it_host = np.zeros((NCORES, NB, 128, Tmax * 128), np.float16)
    for c in range(NCORES):
        for b in range(NB):
            g = c * NB + b
            lo, hi = offsets[g], offsets[g + 1]
            n = hi - lo
            s = np.zeros(EPAD, np.int64)          # pad src = 0 (harmless row)
            s[:n] = src_s[lo:hi]
            idx_host[c, :, b * Tmax * 8:(b + 1) * Tmax * 8] = _wrap_idx(s)
            dr = (dst_s[lo:hi] - g * 128).astype(np.int64)
            pos = np.arange(n)
            t_i = pos // 128
            e_i = pos % 128
            i_host[c, b, e_i, t_i * 128 + dr] = 1.0
            it_host[c, b, dr, t_i * 128 + e_i] = 1.0
    return Tmax, NGRP, idx_host, i_host, it_host, a_host


# ---------------------------------------------------------------------------
def _build_nc(Tmax, NGRP):
    import concourse.bacc as bacc
    import concourse.mybir as mybir
    import concourse.tile as tile

    _apply_tile_drain_patch()
    AO = mybir.AluOpType
    AF = mybir.ActivationFunctionType
    dt = mybir.dt.float32
    f16 = mybir.dt.float16

    nc = bacc.Bacc("TRN2")

    # ---- inputs ----
    xT = nc.dram_tensor("xT", [128, NPAD], f16, kind="ExternalInput")
    xdstT = nc.dram_tensor("xdstT", [128, NODES_PER_CORE], f16,
                           kind="ExternalInput")
    ident_in = nc.dram_tensor("ident_in", [128, 128], f16, kind="ExternalInput")
    gidx = nc.dram_tensor("gidx", [128, NB * Tmax * 8], mybir.dt.int16,
                          kind="ExternalInput")
    i_in = nc.dram_tensor("i_in", [NB, 128, Tmax * 128], f16,
                          kind="ExternalInput")
    it_in = nc.dram_tensor("it_in", [NB, 128, Tmax * 128], f16,
                           kind="ExternalInput")
    a_gcn = nc.dram_tensor("a_gcn", [NCHUNK, 128, NB * 128], f16,
                           kind="ExternalInput")
    b_g1_c = nc.dram_tensor("b_g1_c", [HID, 1], dt, kind="ExternalInput")
    b_g2_c = nc.dram_tensor("b_g2_c", [OUT_CH, 1], dt, kind="ExternalInput")
    w_gat = nc.dram_tensor("w_gat", [IN_CH, HC], f16, kind="ExternalInput")
    wad = nc.dram_tensor("wad", [IN_CH, 16], f16, kind="ExternalInput")
    wd8 = nc.dram_tensor("wd8", [IN_CH, HEADS], f16, kind="ExternalInput")
    w_emb = nc.dram_tensor("w_emb", [HC, HID], f16, kind="ExternalInput")
    w_g1 = nc.dram_tensor("w_g1", [HID, HID], f16, kind="ExternalInput")
    w_g2 = nc.dram_tensor("w_g2", [HID, OUT_CH], f16, kind="ExternalInput")
    b_gat_r = nc.dram_tensor("b_gat_r", [1, HC], dt, kind="ExternalInput")
    b_emb_r = nc.dram_tensor("b_emb_r", [1, HID], dt, kind="ExternalInput")
    out = nc.dram_tensor("out", [NODES_PER_CORE, OUT_CH], dt,
                         kind="ExternalOutput")

    gsem = nc.alloc_semaphore("gsem")
    gcount = [0]  # completed dma_gather calls so far

    RG = [list(range(NCORES))]

    def tiles_of(g):
        return min(GRP, Tmax - g * GRP)

    with tile.TileContext(nc) as tc:
        with (
            tc.tile_pool(name="const", bufs=1) as const,
            tc.tile_pool(name="dram", bufs=1, space="DRAM") as dram,
        ):
            # ---- DRAM scratch ----
            gat_table = dram.tile([NPAD, ROW], f16)
            h1T_loc = dram.tile([128, NODES_PER_CORE], f16)
            h1T_full = dram.tile([NCORES * 128, NODES_PER_CORE], f16)
            h2T_loc = dram.tile([128, NODES_PER_CORE], f16)
            h2T_full = dram.tile([NCORES * 128, NODES_PER_CORE], f16)

            # ---- constants in SBUF ----
            ident = const.tile([128, 128], f16)
            nc.sync.dma_start(out=ident[:], in_=ident_in[:])
            wgat_t = const.tile([IN_CH, HC], f16)
            nc.sync.dma_start(out=wgat_t[:], in_=w_gat[:])
            wad_t = const.tile([IN_CH, 16], f16)
            nc.sync.dma_start(out=wad_t[:], in_=wad[:])
            wd8_t = const.tile([IN_CH, HEADS], f16)
            nc.sync.dma_start(out=wd8_t[:], in_=wd8[:])
            wemb_t = const.tile([HC // 4, 4, HID], f16)   # [128, 4, 64] chunks
            nc.sync.dma_start(
                out=wemb_t[:],
                in_=w_emb[:].rearrange("(k p) f -> p k f", p=128),
            )
            wg1_t = const.tile([HID, HID], f16)
            nc.sync.dma_start(out=wg1_t[:], in_=w_g1[:])
            wg2_t = const.tile([HID, OUT_CH], f16)
            nc.sync.dma_start(out=wg2_t[:], in_=w_g2[:])
            # bias rows broadcast to 128 partitions (fp32)
            bgat_b = const.tile([128, HC], dt)
            r0 = const.tile([1, HC], dt, tag="r0")
            nc.sync.dma_start(out=r0[:], in_=b_gat_r[:])
            nc.gpsimd.partition_broadcast(bgat_b[:], r0[:1, :])
            bemb_b = const.tile([128, HID], dt)
            r1 = const.tile([1, HID], dt, tag="r1")
            nc.sync.dma_start(out=r1[:], in_=b_emb_r[:])
            nc.gpsimd.partition_broadcast(bemb_b[:], r1[:1, :])
            bg1_c = const.tile([HID, 1], dt, tag="bg1c")
            nc.sync.dma_start(out=bg1_c[:], in_=b_g1_c[:])
            bg2_c = const.tile([OUT_CH, 1], dt, tag="bg2c")
            nc.sync.dma_start(out=bg2_c[:], in_=b_g2_c[:])
            # resident per-core metadata
            gidx_sb = const.tile([128, NB * Tmax * 8], mybir.dt.int16,
                                 tag="gi")
            nc.sync.dma_start(out=gidx_sb[:], in_=gidx[:])
            # a_dst for own dst windows [128, NB*8] fp16
            adst_all = const.tile([128, NB * HEADS], f16, tag="adst")
            # shared SBUF copy of the allgathered transposed features
            hT_sb = const.tile([128, NCORES, NODES_PER_CORE], f16, tag="hT")
            # GCN h@W table, SBUF-resident
            tbl_sb = const.tile([128, NCHUNK, HID], f16, tag="tbl")

            def elu_inplace(pool, tile_ap, w, dtype, pdim=128):
                """tile_ap [pdim, w] <- elu(tile_ap); uses pool scratch."""
                xm = pool.tile([pdim, w], dtype, tag=f"elu{w}")
                nc.vector.tensor_scalar(out=xm[:], in0=tile_ap, scalar1=0.0,
                                        scalar2=None, op0=AO.min)
                nc.scalar.activation(out=xm[:], in_=xm[:], func=AF.Exp)
                nc.vector.tensor_scalar(out=tile_ap, in0=tile_ap, scalar1=0.0,
                                        scalar2=None, op0=AO.max)
                nc.vector.scalar_tensor_tensor(
                    out=tile_ap, in0=tile_ap, scalar=-1.0, in1=xm[:],
                    op0=AO.add, op1=AO.add)

            # =============================================================
            # Phase 0: build gat_table rows [a_src | xl | pad], a_dst windows
            # =============================================================
            with (
                tc.tile_pool(name="p0c", bufs=1) as p0c,
                tc.tile_pool(name="p0", bufs=3) as p0,
                tc.tile_pool(name="p0ps", bufs=2, space="PSUM") as p0ps,
            ):
                xT_sb = p0c.tile([128, NPAD], f16, tag="xT")
                nc.sync.dma_start(out=xT_sb[:], in_=xT[:])
                xdstT_sb = p0c.tile([128, NODES_PER_CORE], f16, tag="xdT")
                nc.sync.dma_start(out=xdstT_sb[:], in_=xdstT[:])
                for k in range(NCHUNK):
                    xl_ps = p0ps.tile([128, HC], dt, tag="xl", space="PSUM")
                    nc.tensor.matmul(out=xl_ps[:],
                                     lhsT=xT_sb[:, 128 * k:128 * (k + 1)],
                                     rhs=wgat_t[:], start=True, stop=True)
                    aw_ps = p0ps.tile([128, 16], dt, tag="aw", space="PSUM")
                    nc.tensor.matmul(out=aw_ps[:],
                                     lhsT=xT_sb[:, 128 * k:128 * (k + 1)],
                                     rhs=wad_t[:], start=True, stop=True)
                    row = p0.tile([128, 8 + HC], f16, tag="row")
                    nc.scalar.copy(out=row[:, 0:8], in_=aw_ps[:, 0:8])
                    nc.vector.tensor_copy(out=row[:, 8:8 + 256],
                                          in_=xl_ps[:, 0:256])
                    nc.scalar.copy(out=row[:, 8 + 256:8 + HC],
                                   in_=xl_ps[:, 256:HC])
                    nc.sync.dma_start(
                        out=gat_table[128 * k:128 * (k + 1), 0:8 + HC],
                        in_=row[:])
                # a_dst for own windows, from xdstT
                for b in range(NB):
                    ad_ps = p0ps.tile([128, HEADS], dt, tag="aw", space="PSUM")
                    nc.tensor.matmul(out=ad_ps[:],
                                     lhsT=xdstT_sb[:, 128 * b:128 * (b + 1)],
                                     rhs=wd8_t[:], start=True, stop=True)
                    nc.scalar.copy(
                        out=adst_all[:, b * HEADS:(b + 1) * HEADS],
                        in_=ad_ps[:])

            # =============================================================
            # gather: one call per dst block (all Tmax*128 edge rows).
            # The critical section [gather; wait] occupies gpsimd only, so
            # when it is emitted AFTER a block's compute, its descriptor
            # generation + DMA overlap that compute.
            # =============================================================
            def issue_gather(gpool, table, b, row_w, gtag, defer=True):
                gtile = gpool.tile([128, Tmax, row_w], f16, tag=gtag)
                idx0 = b * Tmax * 8
                with tc.tile_critical(no_gpsimd_drain=True):
                    nc.gpsimd.dma_gather(
                        gtile[:], table[:],
                        gidx_sb[:, idx0:idx0 + Tmax * 8],
                        Tmax * 128, Tmax * 128, row_w,
                        single_packet=False,
                    ).then_inc(gsem, 16)
                    if defer:
                        # run descgen+DMA concurrently with previously
                        # emitted compute; ordering vs the buffer's prior
                        # readers is guaranteed transitively by the
                        # previous section's marker (Pool FIFO).
                        tc.wait_critical_data_deps()
                    gcount[0] += 1
                    nc.gpsimd.wait_ge(gsem, 16 * gcount[0])
                return gtile

            # =============================================================
            # Phase 1: GAT blocks -> h1T_loc [128, 1280] fp16 (rows 0:64)
            # =============================================================
            with (
                tc.tile_pool(name="pg", bufs=2) as pg,       # gathered rows
                tc.tile_pool(name="pt", bufs=2) as pt,       # per-group scratch
                tc.tile_pool(name="pb", bufs=2) as pb,       # per-block scratch
                tc.tile_pool(name="ps_acc", bufs=2, space="PSUM") as ps_acc,
                tc.tile_pool(name="ps_ed", bufs=2, space="PSUM") as ps_ed,
                tc.tile_pool(name="ps_it", bufs=2, space="PSUM") as ps_it,
            ):
                gt_next = issue_gather(pg, gat_table, 0, ROW, "g",
                                       defer=False)
                for b in range(NB):
                    gt = gt_next
                    Ib = pt.tile([128, Tmax * 128], f16, tag="Ib")
                    nc.sync.dma_start(out=Ib[:], in_=i_in[b])
                    ITb = pt.tile([128, Tmax * 128], f16, tag="ITb")
                    nc.sync.dma_start(out=ITb[:], in_=it_in[b])
                    msg_ps = ps_acc.tile([128, HC], dt, tag="msg",
                                         space="PSUM")
                    aux_ps = ps_acc.tile([128, 128], dt, tag="aux",
                                         space="PSUM")
                    adw = adst_all[:, b * HEADS:(b + 1) * HEADS]
                    for g in range(NGRP):
                        t = tiles_of(g)
                        t0 = g * GRP
                        gts = gt[:, t0:t0 + t, :]
                        # ed[e, th] = a_dst[dstrel_e, h] via host-built IT
                        ed_ps = ps_ed.tile([128, GRP * HEADS], dt, tag="ed",
                                           space="PSUM")
                        for tm in range(t):
                            tt = t0 + tm
                            nc.tensor.matmul(
                                out=ed_ps[:, tm * HEADS:(tm + 1) * HEADS],
                                lhsT=ITb[:, tt * 128:(tt + 1) * 128],
                                rhs=adw, start=True, stop=True)
                        # el = leaky(a_src + ed); p = exp(el)
                        el = pt.tile([128, GRP * HEADS], f16, tag="el")
                        nc.vector.tensor_tensor(
                            out=el[:, 0:t * HEADS].rearrange(
                                "p (t h) -> p t h", t=t),
                            in0=gts[:, :, 0:8],
                            in1=ed_ps[:, 0:t * HEADS].rearrange(
                                "p (t h) -> p t h", t=t),
                            op=AO.add)
                        nc.vector.scalar_tensor_tensor(
                            out=el[:, 0:t * HEADS], in0=el[:, 0:t * HEADS],
                            scalar=NEG_SLOPE, in1=el[:, 0:t * HEADS],
                            op0=AO.mult, op1=AO.max)
                        p_blk = pt.tile([128, GRP, HEADS], f16, tag="p")
                        nc.scalar.activation(
                            out=p_blk[:, 0:t, :],
                            in_=el[:, 0:t * HEADS].rearrange(
                                "p (t h) -> p t h", t=t),
                            func=AF.Exp)
                        # alpha * xl: heads 0:4 via ACT-broadcast + 2x mult,
                        # heads 4:8 via direct DVE broadcast mult
                        HH = HEADS // 2
                        p_brd = pt.tile([128, GRP, HH, HID], f16, tag="pb")
                        nc.scalar.copy(
                            out=p_brd[:, 0:t],
                            in_=p_blk[:, 0:t, 0:HH].to_broadcast(
                                [128, t, HH, HID]))
                        msg_g = pt.tile([128, GRP, HC], f16, tag="m")
                        nc.vector.tensor_tensor(
                            out=msg_g[:, 0:t, 0:HH * HID].rearrange(
                                "p t (h c) -> p t h c", h=HH),
                            in0=gts[:, :, 8:8 + HH * HID].rearrange(
                                "p t (h c) -> p t h c", h=HH),
                            in1=p_brd[:, 0:t],
                            op=AO.mult)
                        nc.vector.tensor_tensor(
                            out=msg_g[:, 0:t, HH * HID:HC].rearrange(
                                "p t (h c) -> p t h c", h=HH),
                            in0=gts[:, :, 8 + HH * HID:8 + HC].rearrange(
                                "p t (h c) -> p t h c", h=HH),
                            in1=p_blk[:, 0:t, HH:HEADS].to_broadcast(
                                [128, t, HH, HID]),
                            op=AO.mult)
                        for tm in range(t):
                            tt = t0 + tm
                            nc.tensor.matmul(
                                out=msg_ps[:],
                                lhsT=Ib[:, tt * 128:(tt + 1) * 128],
                                rhs=msg_g[:, tm, :],
                                start=(tt == 0), stop=(tt == Tmax - 1))
                            nc.tensor.matmul(
                                out=aux_ps[:, 0:8],
                                lhsT=Ib[:, tt * 128:(tt + 1) * 128],
                                rhs=p_blk[:, tm, :],
                                start=(tt == 0), stop=(tt == Tmax - 1))
                    # ---- block epilogue ----
                    sinv = pb.tile([128, HEADS], dt, tag="sinv")
                    nc.vector.tensor_scalar(out=sinv[:], in0=aux_ps[:, 0:8],
                                            scalar1=1e-30, scalar2=None,
                                            op0=AO.max)
                    nc.vector.reciprocal(out=sinv[:], in_=sinv[:])
                    o = pb.tile([128, HC], f16, tag="o")
                    nc.vector.tensor_tensor(
                        out=o[:].rearrange("p (h c) -> p h c", h=HEADS),
                        in0=msg_ps[:].rearrange("p (h c) -> p h c", h=HEADS),
                        in1=sinv[:].to_broadcast([128, HEADS, HID]),
                        op=AO.mult)
                    nc.vector.tensor_tensor(out=o[:], in0=o[:], in1=bgat_b[:],
                                            op=AO.add)
                    elu_inplace(pb, o[:], HC, f16)
                    # emb: h1 = elu(o @ w_emb + b_emb)
                    emb_ps = aux_ps[:, 64:128]
                    for kk in range(4):
                        ht_ps = ps_it.tile([128, 128], f16, tag="it",
                                           space="PSUM")
                        nc.tensor.transpose(out=ht_ps[:],
                                            in_=o[:, 128 * kk:128 * (kk + 1)],
                                            identity=ident[:])
                        ht = pb.tile([128, 128], f16, tag="ht")
                        nc.scalar.copy(out=ht[:], in_=ht_ps[:])
                        nc.tensor.matmul(out=emb_ps, lhsT=ht[:],
                                         rhs=wemb_t[:, kk, :],
                                         start=(kk == 0), stop=(kk == 3))
                    h1b = pb.tile([128, HID], f16, tag="h1b")
                    nc.vector.tensor_tensor(out=h1b[:], in0=emb_ps,
                                            in1=bemb_b[:], op=AO.add)
                    elu_inplace(pb, h1b[:], HID, f16)
                    # transpose; store to h1T_loc block cols
                    hd_ps = ps_it.tile([128, 128], f16, tag="it",
                                       space="PSUM")
                    nc.tensor.transpose(out=hd_ps[0:HID, :], in_=h1b[:],
                                        identity=ident[:])
                    hdT = pb.tile([HID, 128], f16, tag="hdT")
                    nc.scalar.copy(out=hdT[:], in_=hd_ps[0:HID, :])
                    nc.sync.dma_start(
                        out=h1T_loc[0:HID, 128 * b:128 * (b + 1)], in_=hdT[:])
                    # prefetch next block's gather AFTER this block's compute
                    # so descgen+DMA overlap it
                    if b + 1 < NB:
                        gt_next = issue_gather(pg, gat_table, b + 1, ROW, "g")

            # =============================================================
            # GCN layers: dense A^T blocks streamed from DRAM
            # =============================================================
            nc.gpsimd.collective_compute(
                "AllGather", mybir.AluOpType.bypass, replica_groups=RG,
                ins=[h1T_loc.opt()], outs=[h1T_full.opt()])

            def gcn_layer(hT_full, w_t, bias_col, h_out_loc, is_last):
                # table rows = h[n] @ W into SBUF (tbl_sb), from gathered hT
                nc.sync.dma_start(
                    out=hT_sb[:],
                    in_=hT_full[:].rearrange("(s p) f -> p s f", p=128))
                with (
                    tc.tile_pool(name="q0ps", bufs=2, space="PSUM") as q0ps,
                ):
                    for k in range(NCHUNK):
                        c, blk = divmod(k, NB)
                        t_ps = q0ps.tile([128, HID], dt, tag="t",
                                         space="PSUM")
                        nc.tensor.matmul(
                            out=t_ps[:],
                            lhsT=hT_sb[0:HID, c, 128 * blk:128 * (blk + 1)],
                            rhs=w_t[:], start=True, stop=True)
                        nc.scalar.copy(out=tbl_sb[:, k, :], in_=t_ps[:])
                # dense accumulation over all chunks, transposed output
                # accT [HID, 1280] = sum_c tbl_c^T @ A^T_c
                ND = NB * 128
                with (
                    tc.tile_pool(name="qa", bufs=3) as qa,
                    tc.tile_pool(name="qb", bufs=1) as qb,
                    tc.tile_pool(name="qps", bufs=1, space="PSUM") as qps,
                    tc.tile_pool(name="qps2", bufs=2, space="PSUM") as qps2,
                ):
                    spans = [(0, 512), (512, 1024), (1024, ND)]
                    accs = []
                    for si, (lo, hi) in enumerate(spans):
                        acc = qps.tile([HID, hi - lo], dt, tag=f"acc{si}",
                                       name=f"acc{si}", space="PSUM")
                        accs.append(acc)
                    for c0 in range(0, NCHUNK, 8):
                        cn = min(8, NCHUNK - c0)
                        a_sb = qa.tile([128, 8, ND], f16, tag="a")
                        nc.sync.dma_start(
                            out=a_sb[:, 0:cn, :],
                            in_=a_gcn[c0:c0 + cn].rearrange(
                                "c p d -> p c d"))
                        for ci in range(cn):
                            c = c0 + ci
                            for si, (lo, hi) in enumerate(spans):
                                nc.tensor.matmul(
                                    out=accs[si][:], lhsT=tbl_sb[:, c, :],
                                    rhs=a_sb[:, ci, lo:hi],
                                    start=(c == 0), stop=(c == NCHUNK - 1))
                    # epilogue in transposed layout [HID, 1280]
                    obT = qb.tile([HID, ND], f16, tag="obT")
                    for si, (lo, hi) in enumerate(spans):
                        nc.vector.tensor_scalar(
                            out=obT[:, lo:hi], in0=accs[si][:],
                            scalar1=bias_col, scalar2=None, op0=AO.add)
                    if is_last:
                        # transpose back per block, write node-major f32
                        for b in range(NB):
                            ob_ps = qps2.tile([128, OUT_CH], f16, tag="obp",
                                              space="PSUM")
                            nc.tensor.transpose(
                                out=ob_ps[:],
                                in_=obT[:, 128 * b:128 * (b + 1)],
                                identity=ident[0:OUT_CH, 0:OUT_CH])
                            obf = qb.tile([128, OUT_CH], dt, tag="obf")
                            nc.scalar.copy(out=obf[:], in_=ob_ps[:])
                            nc.sync.dma_start(
                                out=h_out_loc[128 * b:128 * (b + 1)],
                                in_=obf[:])
                    else:
                        elu_inplace(qb, obT[:], ND, f16, pdim=HID)
                        nc.sync.dma_start(
                            out=h_out_loc[0:HID, :], in_=obT[:])

            gcn_layer(h1T_full, wg1_t, bg1_c[:], h2T_loc, False)
            nc.gpsimd.collective_compute(
                "AllGather", mybir.AluOpType.bypass, replica_groups=RG,
                ins=[h2T_loc.opt()], outs=[h2T_full.opt()])
            gcn_layer(h2T_full, wg2_t, bg2_c[:], out, True)

    nc.finalize()
    return nc


# ---------------------------------------------------------------------------
def _run(inputs, trace=False, **run_kw):
    from concourse import bass_utils

    x = np.asarray(inputs["x"], np.float32)
    edge_index = np.asarray(inputs["edge_index"])
    W_gat = np.asarray(inputs["W_gat"], np.float32)
    att_src = np.asarray(inputs["att_src"], np.float32)
    att_dst = np.asarray(inputs["att_dst"], np.float32)
    b_gat = np.asarray(inputs["b_gat"], np.float32)
    W_emb = np.asarray(inputs["W_emb"], np.float32)
    b_emb = np.asarray(inputs["b_emb"], np.float32)
    W_g1 = np.asarray(inputs["W_g1"], np.float32)
    b_g1 = np.asarray(inputs["b_g1"], np.float32)
    W_g2 = np.asarray(inputs["W_g2"], np.float32)
    b_g2 = np.asarray(inputs["b_g2"], np.float32)

    Tmax, NGRP, idx_host, i_host, it_host, a_host = _host_prep(edge_index)
    nc = _build_nc(Tmax, NGRP)

    x_big = np.zeros((NCORES * NODES_PER_CORE, IN_CH), np.float32)
    x_big[:N] = x
    xT = x_big[:NPAD].T.astype(np.float16).copy()
    # Wa[k,h] = sum_c W_gat[k, h*HID+c] * att_src[h,c]  (same for Wd/att_dst)
    Wr = W_gat.reshape(IN_CH, HEADS, HID)
    Wa = np.einsum("khc,hc->kh", Wr, att_src).astype(np.float16)
    Wd = np.einsum("khc,hc->kh", Wr, att_dst).astype(np.float16)

    common = {
        "xT": xT,
        "ident_in": np.eye(128, dtype=np.float16),
        "w_gat": W_gat.astype(np.float16),
        "wad": np.concatenate([Wa, Wd], axis=1),
        "wd8": Wd,
        "w_emb": W_emb.astype(np.float16),
        "w_g1": W_g1.astype(np.float16),
        "w_g2": W_g2.astype(np.float16),
        "b_gat_r": b_gat[None, :], "b_emb_r": b_emb[None, :],
        "b_g1_r": b_g1[None, :], "b_g2_r": b_g2[None, :],
        "b_g1_c": b_g1[:, None].copy(), "b_g2_c": b_g2[:, None].copy(),
    }
    in_maps = []
    for c in range(NCORES):
        m = dict(common)
        m["xdstT"] = (
            x_big[c * NODES_PER_CORE:(c + 1) * NODES_PER_CORE].T
            .astype(np.float16).copy())
        m["gidx"] = idx_host[c]
        m["i_in"] = i_host[c]
        m["it_in"] = it_host[c]
        m["a_gcn"] = a_host[c]
        in_maps.append(m)

    res = bass_utils.run_bass_kernel_spmd(
        nc, in_maps, core_ids=list(range(NCORES)), trace=trace, **run_kw)
    full = np.concatenate([res.results[c]["out"] for c in range(NCORES)],
                          axis=0)
    return full[:N], res


def kernel(**inputs) -> np.ndarray:
    out, _ = _run(inputs, trace=False)
    return out
